# revision 33
# baseline (speedup 1.0000x reference)
"""Trainium2 Bass kernel for nn_LinearEncoder (2-layer GCN + dense branch).

Strategy (8 NeuronCores, SPMD):
  - Nodes are degree-sorted and dealt round-robin to 8 cores (load balance);
    each core owns PPC=12544 destination positions (98 blocks of 128).
  - GCN linearity: aggregate the 128-wide scaled node table u = x*dinv
    (resp. z = (g1@Wg2)*dinv for layer 2), then apply the weight matmul once.
  - Layer-1 table u is host-computed and replicated to every core as an
    input (no AllGather); the layer-2 z table is computed on-device and
    AllGather'd (bf16). Each core gathers its incoming-edge source rows
    with batched dma_gather (4096-idx instructions, int16 chunk-relative
    indices, 4 SWDGE queues) and segment-sums them on the TensorEngine via
    one-hot matmuls accumulated in PSUM (feat-major), superblock by
    superblock. Self-loops skip the gather: they are added straight into
    PSUM by identity-matrix matmuls over the local (dinv-scaled) table.
  - Dense branches are feat-major bf16 matmuls with biases/ReLU on the
    Scalar (ACT) engine; outputs are written transposed per core and
    un-permuted on the host.
"""

import numpy as np
import ml_dtypes

import concourse.bacc as bacc
import concourse.mybir as mybir
import concourse.tile as tile
from concourse import bass_utils

F32 = mybir.dt.float32
BF16 = mybir.dt.bfloat16
I16 = mybir.dt.int16
I32 = mybir.dt.int32
NEG = -1.0  # dstl mask value


class CFG:
    def __init__(self, N, ncores=8, ch_rows=32768, nidx=4096, sb_blocks=20):
        self.N = N
        self.ncores = ncores
        per = -(-N // ncores)
        self.per = per                      # real nodes per core (first cores)
        self.ppc = -(-per // 128) * 128     # padded per core
        self.nb = self.ppc // 128           # blocks per core
        self.trows = ncores * self.ppc      # real table rows
        self.ch = ch_rows
        self.nchunk = -(-self.trows // ch_rows)
        self.trows_pad = self.nchunk * ch_rows
        self.nidx = nidx
        self.gq = 4  # gather SWDGE queues (set 1 for CoreSim validation)
        # superblock partition of blocks
        sbs = []
        b = self.nb
        while b > 0:
            sbs.append(min(sb_blocks, b))
            b -= min(sb_blocks, b)
        self.sbs = sbs


def _deal_nodes(deg, cfg):
    """Degree-sorted round-robin deal of nodes to (core, pos)."""
    N = cfg.N
    order = np.argsort(-deg, kind="stable")
    core_of = np.empty(N, np.int64)
    pos_of = np.empty(N, np.int64)
    r = np.arange(N, dtype=np.int64)
    core_of[order] = r % cfg.ncores
    pos_of[order] = r // cfg.ncores
    return core_of, pos_of


def build_schedule(edge_index, cfg):
    """Static SPMD schedule + per-core device arrays, from the actual graph."""
    N, K = cfg.N, cfg.ncores
    src = np.asarray(edge_index[0], dtype=np.int64)
    dst = np.asarray(edge_index[1], dtype=np.int64)
    deg = np.bincount(dst, minlength=N).astype(np.int64) + 1
    dinv = (1.0 / np.sqrt(deg.astype(np.float64))).astype(np.float32)

    core_of, pos_of = _deal_nodes(deg, cfg)
    row_of = core_of * cfg.ppc + pos_of  # table row of each node

    # per-core real node counts
    npercore = np.bincount(core_of, minlength=K)

    # real edges only; self-loops are added on-device via identity matmuls
    esrc = src
    edst = dst

    e_core = core_of[edst]
    e_pos = pos_of[edst]
    e_blk = e_pos >> 7
    e_p = (e_pos & 127).astype(np.float32)
    e_rowsrc = row_of[esrc]
    e_ch = e_rowsrc // cfg.ch
    e_rel = (e_rowsrc % cfg.ch).astype(np.int16)

    nb, nch = cfg.nb, cfg.nchunk
    sb_of_blk = np.repeat(np.arange(len(cfg.sbs)), cfg.sbs)

    # group = (sb, ch, blk); order edges by (core, sb, ch, blk)
    g_of_e = (sb_of_blk[e_blk] * nch + e_ch) * nb + e_blk  # group id within core
    ngrp_ids = len(cfg.sbs) * nch * nb  # sparse (blk implies sb) but fine
    key = e_core * ngrp_ids + g_of_e
    eord = np.argsort(key, kind="stable")
    key_s = key[eord]

    # counts per (core, group)
    cnt = np.bincount(key_s, minlength=K * ngrp_ids).reshape(K, ngrp_ids)

    # group list in slot order: for sb, for ch, for blk in sb
    grp_list = []  # (sb, ch, blk, gid)
    for sbi, sbn in enumerate(cfg.sbs):
        blk0 = sum(cfg.sbs[:sbi])
        for ch in range(nch):
            for blk in range(blk0, blk0 + sbn):
                gid = (sbi * nch + ch) * nb + blk
                grp_list.append((sbi, ch, blk, gid))

    # padded group sizes: max over cores; chunk-0 groups at least 1
    gmax = {}
    for sbi, ch, blk, gid in grp_list:
        m = int(cnt[:, gid].max())
        if ch == 0:
            m = max(m, 1)
        gmax[gid] = m

    # per-(sb,ch) runs: pad total to x16; compute slot offsets
    runs = []  # (sbi, ch, slot_off, n_slots, [(blk, gid, off_in_run, gsize)])
    slot_blk_parts = []
    total = 0
    for sbi, sbn in enumerate(cfg.sbs):
        blk0 = sum(cfg.sbs[:sbi])
        for ch in range(nch):
            glist = []
            off = 0
            for blk in range(blk0, blk0 + sbn):
                gid = (sbi * nch + ch) * nb + blk
                gs = gmax[gid]
                if gs:
                    glist.append((blk, gid, off, gs))
                off += gs
            pad_tail = (-off) % 16
            n = off + pad_tail
            sb_slot_blk = np.full(n, -1, np.int64)
            for blk, gid, o, gs in glist:
                sb_slot_blk[o:o + gs] = blk
            runs.append((sbi, ch, total, n, glist))
            slot_blk_parts.append(sb_slot_blk)
            total += n
    n_slots = total
    slot_blk = np.concatenate(slot_blk_parts) if slot_blk_parts else np.zeros(0, np.int64)

    # pad rows per chunk (zero rows of the table): first padded position of
    # some core inside each chunk's row range
    pad_row_rel = np.full(nch, -1, np.int64)
    for c in range(K):
        if npercore[c] < cfg.ppc:
            r0 = c * cfg.ppc + npercore[c]
            ch = r0 // cfg.ch
            if pad_row_rel[ch] < 0:
                pad_row_rel[ch] = r0 % cfg.ch
    # fallback: fill missing chunks with any real zero... must not happen
    for ch in range(nch):
        if pad_row_rel[ch] < 0:
            # point at the last real row of the chunk; its value times a
            # zero one-hot column contributes nothing (dstl = -1 for pads)
            pad_row_rel[ch] = 0

    # per-core slot arrays: idx (int16 rel) + dstl (float p or -1)
    slot_idx = np.zeros((K, n_slots), np.int16)
    slot_dstl = np.full((K, n_slots), NEG, np.float32)
    # default pad idx per run
    for (sbi, ch, off, n, glist) in runs:
        slot_idx[:, off:off + n] = pad_row_rel[ch]
    # place real edges: rank within (core, group)
    grp_off = {}
    for (sbi, ch, off, n, glist) in runs:
        for blk, gid, o, gs in glist:
            grp_off[gid] = off + o
    # vectorized placement
    uk, inv = np.unique(key_s, return_inverse=True)
    starts = np.searchsorted(key_s, uk)
    rank = np.arange(len(key_s)) - starts[inv]
    core_s = key_s // ngrp_ids
    gid_s = key_s % ngrp_ids
    base = np.array([grp_off.get(int(g), -1) for g in uk % ngrp_ids], np.int64)
    slot_pos = base[inv] + rank
    assert (base[inv] >= 0).all()
    slot_idx[core_s, slot_pos] = e_rel[eord]
    slot_dstl[core_s, slot_pos] = e_p[eord]

    # instructions: slice each run into <= nidx pieces
    instrs = []  # (sbi, ch, idx_col_off, n_idx, units)
    # units: (g_tile, grp4, blk, col, start, stop)
    # dstl columns are appended as discovered
    col_count = 0
    unit_cols = []  # (slot_off_of_tile, n_valid, blk) for building dstl cols
    first_seen = {}
    last_seen = {}
    order_units = []
    for (sbi, ch, roff, rn, glist) in runs:
        o = 0
        while o < rn:
            n = min(cfg.nidx, rn - o)
            base_slot = roff + o
            units = []
            ntiles = -(-n // 128)
            for g in range(ntiles):
                t0 = base_slot + g * 128
                t1 = min(t0 + 128, base_slot + n)
                blks = np.unique(slot_blk[t0:t1])
                for blk in blks:
                    if blk < 0:
                        continue
                    col = col_count
                    col_count += 1
                    unit_cols.append((t0, t1 - t0, int(blk)))
                    u = [g, int(blk), col]
                    units.append(u)
                    kkey = (sbi, int(blk))
                    if kkey not in first_seen:
                        first_seen[kkey] = (len(instrs), len(units) - 1)
                    last_seen[kkey] = (len(instrs), len(units) - 1)
            instrs.append([sbi, ch, base_slot, n, units])
            o += n
    # mark start/stop
    for ii, (sbi, ch, base_slot, n, units) in enumerate(instrs):
        for ui, (g, blk, col) in enumerate(units):
            st = first_seen[(sbi, blk)] == (ii, ui)
            sp = last_seen[(sbi, blk)] == (ii, ui)
            units[ui] = (g, blk, col, st, sp)

    # per-core dstl column array [128, col_count]
    dstl_cols = np.full((K, 128, col_count), NEG, np.float32)
    for col, (t0, nvalid, blk) in enumerate(unit_cols):
        seg = slot_dstl[:, t0:t0 + nvalid]           # [K, nvalid]
        segblk = slot_blk[t0:t0 + nvalid]            # [nvalid]
        m = segblk == blk
        v = np.where(m[None, :], seg, NEG)
        dstl_cols[:, :nvalid, col] = v

    # per-core idx array wrapped per instruction: [128, n_slots/16]
    assert n_slots % 16 == 0
    idx_arr = np.zeros((K, 128, n_slots // 16), np.int16)
    for (sbi, ch, base_slot, n, units) in instrs:
        seg = slot_idx[:, base_slot:base_slot + n]    # [K, n]
        assert n % 16 == 0
        w = seg.reshape(K, n // 16, 16).transpose(0, 2, 1)  # [K,16,n/16]
        c0 = base_slot // 16
        for grp in range(8):
            idx_arr[:, 16 * grp:16 * grp + 16, c0:c0 + n // 16] = w

    return dict(
        deg=deg, dinv=dinv, core_of=core_of, pos_of=pos_of, row_of=row_of,
        npercore=npercore, instrs=instrs, n_slots=n_slots, col_count=col_count,
        idx_arr=idx_arr, dstl_cols=dstl_cols, runs=runs,
    )


def build_program(cfg, sched):
    nc = bacc.Bacc("TRN2", target_bir_lowering=False, debug=False,
                   num_devices=cfg.ncores, num_swdge_queues=cfg.gq)
    PPC, NB, CH, NCH = cfg.ppc, cfg.nb, cfg.ch, cfg.nchunk
    D = 128

    # ---- inputs ----
    utable = nc.dram_tensor("utable", [cfg.trows_pad, D], BF16, kind="ExternalInput")
    uTf = nc.dram_tensor("uTf", [D, PPC], BF16, kind="ExternalInput")
    xnT = nc.dram_tensor("xnT", [D, PPC], BF16, kind="ExternalInput")
    xsT = nc.dram_tensor("xsT", [D, PPC], BF16, kind="ExternalInput")
    dinvb = nc.dram_tensor("dinvb", [D, PPC], F32, kind="ExternalInput")
    dinvc = nc.dram_tensor("dinvc", [D, NB], F32, kind="ExternalInput")
    idx_d = nc.dram_tensor("idx", [D, sched["n_slots"] // 16], I16, kind="ExternalInput")
    dstl_d = nc.dram_tensor("dstl", [D, sched["col_count"]], BF16, kind="ExternalInput")
    w_ins = nc.dram_tensor("W_in_self", [D, 256], BF16, kind="ExternalInput")
    w_os = nc.dram_tensor("W_out_self", [384, D], BF16, kind="ExternalInput")
    wg1 = nc.dram_tensor("Wg1", [D, 256], BF16, kind="ExternalInput")
    wg2 = nc.dram_tensor("Wg2", [256, D], BF16, kind="ExternalInput")
    w_out = nc.dram_tensor("W_out", [512, D], BF16, kind="ExternalInput")
    biases = nc.dram_tensor("biases", [D, 7], F32, kind="ExternalInput")
    # bias cols: 0,1 b_in_self | 2 b_out_self | 3,4 bg1 | 5 bg2 | 6 b_out
    l1_out = nc.dram_tensor("l1T", [D, PPC], F32, kind="ExternalOutput")
    x2_out = nc.dram_tensor("x2T", [D, PPC], F32, kind="ExternalOutput")

    instrs = sched["instrs"]
    MAXU = max(len(u[4]) for u in instrs)

    with tile.TileContext(nc) as tc:
        with tc.tile_pool(name="const", bufs=1) as constp, \
             tc.tile_pool(name="dram", bufs=1, space="DRAM") as dramp, \
             tc.tile_pool(name="idxs", bufs=4) as idxsp, \
             tc.tile_pool(name="stage", bufs=8) as stagep, \
             tc.tile_pool(name="oh", bufs=3) as ohp, \
             tc.tile_pool(name="pagg", bufs=6, space="PSUM") as paggp, \
             tc.tile_pool(name="pdense", bufs=2, space="PSUM") as pdensep, \
             tc.tile_pool(name="hT", bufs=4) as hTp, \
             tc.tile_pool(name="sm", bufs=8) as smp, \
             tc.tile_pool(name="outs", bufs=4) as outsp, \
             tc.tile_pool(name="dinvs", bufs=2) as dinvsp:

            # constants
            iota_i32 = constp.tile([128, 128], I32)
            nc.gpsimd.iota(iota_i32[:], pattern=[[1, 128]], base=0, channel_multiplier=0)
            iota_bf = constp.tile([128, 128], BF16)
            nc.vector.tensor_copy(out=iota_bf[:], in_=iota_i32[:])
            zeros512 = constp.tile([128, 512], BF16)
            nc.vector.memset(zeros512[:], 0.0)
            # identity matrix for self-loop psum adds
            iota_ch = constp.tile([128, 128], I32)
            nc.gpsimd.iota(iota_ch[:], pattern=[[0, 128]], base=0, channel_multiplier=1)
            ident = constp.tile([128, 128], BF16)
            nc.vector.tensor_tensor(out=ident[:], in0=iota_i32[:], in1=iota_ch[:],
                                    op=mybir.AluOpType.is_equal)

            wins_sb = constp.tile([128, 256], BF16)
            nc.sync.dma_start(out=wins_sb[:], in_=w_ins[:, :])
            wos_sb = [constp.tile([128, 128], BF16, tag=f"wos{k}", name=f"wos{k}") for k in range(3)]
            for k in range(3):
                nc.sync.dma_start(out=wos_sb[k][:], in_=w_os[k * 128:(k + 1) * 128, :])
            wg1_sb = constp.tile([128, 256], BF16)
            nc.sync.dma_start(out=wg1_sb[:], in_=wg1[:, :])
            wg2_sb = [constp.tile([128, 128], BF16, tag=f"wg2{k}", name=f"wg2{k}") for k in range(2)]
            for k in range(2):
                nc.sync.dma_start(out=wg2_sb[k][:], in_=wg2[k * 128:(k + 1) * 128, :])
            wout_sb = [constp.tile([128, 128], BF16, tag=f"wo{k}", name=f"wo{k}") for k in range(4)]
            for k in range(4):
                nc.sync.dma_start(out=wout_sb[k][:], in_=w_out[k * 128:(k + 1) * 128, :])
            bias_sb = constp.tile([128, 7], F32)
            nc.sync.dma_start(out=bias_sb[:], in_=biases[:, :])
            dinvc_sb = constp.tile([128, NB], F32)
            nc.sync.dma_start(out=dinvc_sb[:], in_=dinvc[:, :])


            # z-table plumbing (written during L1 phase, AG'd before L2)
            zsh = dramp.tile([PPC, D], BF16)
            ztable = dramp.tile([cfg.trows_pad, D], BF16)
            zTf_d = dramp.tile([D, PPC], BF16)

            # g1T spill (two halves, feat-major)
            g1T_d = [dramp.tile([D, PPC], BF16, name=f"g1T{h}") for h in range(2)]

            # ---------- dense self branch (independent) ----------
            for b in range(NB):
                xs_blk = smp.tile([128, 128], BF16, tag="xs")
                nc.sync.dma_start(out=xs_blk[:], in_=xsT[:, b * 128:(b + 1) * 128])
                l1ps = pdensep.tile([128, 512], F32, tag="pd")
                for h in range(2):
                    nc.tensor.matmul(
                        out=l1ps[:, h * 128:(h + 1) * 128],
                        lhsT=wins_sb[:, h * 128:(h + 1) * 128],
                        rhs=xs_blk[:], start=True, stop=True)
                l1b = smp.tile([128, 256], BF16, tag="l1b")
                for h in range(2):
                    nc.scalar.activation(
                        l1b[:, h * 128:(h + 1) * 128],
                        l1ps[:, h * 128:(h + 1) * 128],
                        mybir.ActivationFunctionType.Relu,
                        bias=bias_sb[:, h:h + 1], scale=1.0)
                o1ps = pdensep.tile([128, 512], F32, tag="pd")
                nc.tensor.matmul(out=o1ps[:, :128], lhsT=wos_sb[0][:], rhs=xs_blk[:],
                                 start=True, stop=False)
                for h in range(2):
                    nc.tensor.matmul(out=o1ps[:, :128], lhsT=wos_sb[1 + h][:],
                                     rhs=l1b[:, h * 128:(h + 1) * 128],
                                     start=False, stop=(h == 1))
                o1 = outsp.tile([128, 128], F32, tag="o1")
                nc.scalar.activation(
                    o1[:], o1ps[:, :128],
                    mybir.ActivationFunctionType.Identity,
                    bias=bias_sb[:, 2:3], scale=1.0)
                nc.sync.dma_start(out=l1_out[:, b * 128:(b + 1) * 128], in_=o1[:])

            # ---------- per-layer aggregation ----------
            def agg_layer(table, selfT_d, layer):
                """Returns nothing; layer==0 computes g1T+z, layer==1 computes g2T+x2."""
                qn = [0]
                ii = 0
                n_instr = len(instrs)
                while ii < n_instr:
                    sbi = instrs[ii][0]
                    blk0 = sum(cfg.sbs[:sbi])
                    sbn = cfg.sbs[sbi]
                    ngrp = -(-sbn // 4)
                    # psum group tiles for this superblock
                    gtiles = [paggp.tile([128, 512], F32, tag="agg", name=f"agg_{layer}_{sbi}_{gg}") for gg in range(ngrp)]
                    for gt in gtiles:
                        nc.tensor.matmul(out=gt[:], lhsT=iota_bf[:], rhs=zeros512[:],
                                         start=True, stop=False)
                    # dinvb slice for this superblock
                    dv = dinvsp.tile([128, sbn * 128], F32, tag="dv")
                    nc.sync.dma_start(
                        out=dv[:], in_=dinvb[:, blk0 * 128:(blk0 + sbn) * 128])
                    # self-loop contributions (identity matmul into psum)
                    sfT = dinvsp.tile([128, sbn * 128], BF16, tag="sfT")
                    nc.scalar.dma_start(
                        out=sfT[:], in_=selfT_d[:, blk0 * 128:(blk0 + sbn) * 128])
                    for gi in range(ngrp):
                        w = min(4, sbn - gi * 4)
                        nc.tensor.matmul(
                            out=gtiles[gi][:, :w * 128], lhsT=ident[:],
                            rhs=sfT[:, gi * 512:gi * 512 + w * 128],
                            start=False, stop=False, skip_group_check=True)
                    # all instructions of this superblock (4 chunks)
                    while ii < n_instr and instrs[ii][0] == sbi:
                        _, ch, base_slot, n, units = instrs[ii]
                        idx_sb_t = idxsp.tile([128, -(-cfg.nidx // 16)], I16, tag="idx")
                        ncols16 = n // 16
                        nc.scalar.dma_start(
                            out=idx_sb_t[:, :ncols16],
                            in_=idx_d[:, base_slot // 16: base_slot // 16 + ncols16])
                        stage = stagep.tile([128, cfg.nidx], BF16, tag="stage")
                        nc.gpsimd.dma_gather(
                            out_ap=stage[:, :(-(-n // 128)) * 128].rearrange(
                                "p (g e) -> p g e", e=D),
                            in_ap=table[ch * CH:(ch + 1) * CH, :],
                            idxs_ap=idx_sb_t[:, :ncols16],
                            num_idxs=n, num_idxs_reg=n, elem_size=D,
                            single_packet=False, queue_num=qn[0] % cfg.gq)
                        qn[0] += 1
                        nu = len(units)
                        oh = ohp.tile([128, MAXU * 128], BF16, tag="oh")
                        c0 = units[0][2]
                        assert units[-1][2] - c0 + 1 == nu
                        nc.vector.tensor_tensor(
                            out=oh[:, :nu * 128].rearrange("p (u e) -> p u e", e=128),
                            in0=iota_bf[:].rearrange("p (a e) -> p a e", a=1)
                                .to_broadcast([128, nu, 128]),
                            in1=dstl_sb[:, c0:c0 + nu].to_broadcast([128, nu, 128]),
                            op=mybir.AluOpType.is_equal)
                        stage3 = stage[:].rearrange("p (g e) -> p g e", e=D)
                        for (g, blk, col, st, sp) in units:
                            gi = (blk - blk0) // 4
                            sl = (blk - blk0) % 4
                            kk = min(128, n - g * 128)
                            nc.tensor.matmul(
                                out=gtiles[gi][:, sl * 128:(sl + 1) * 128],
                                lhsT=stage3[:kk, g, :],
                                rhs=oh[:kk, (col - c0) * 128:(col - c0 + 1) * 128],
                                start=False, stop=False, skip_group_check=True)
                        ii += 1
                    for gt in gtiles:
                        nc.tensor.matmul(out=gt[:], lhsT=iota_bf[:], rhs=zeros512[:],
                                         start=False, stop=True)
                    # final pass per group + per-block dense work
                    for gi in range(ngrp):
                        w = min(4, sbn - gi * 4)
                        hT = hTp.tile([128, 512], BF16, tag="hT")
                        dslice = dv[:, gi * 4 * 128:(gi * 4 + w) * 128]
                        if layer == 0:
                            nc.vector.tensor_tensor(
                                out=hT[:, :w * 128], in0=gtiles[gi][:, :w * 128],
                                in1=dslice, op=mybir.AluOpType.mult)
                        else:
                            g2f = hTp.tile([128, 512], F32, tag="g2f")
                            nc.vector.tensor_tensor(
                                out=g2f[:, :w * 128], in0=gtiles[gi][:, :w * 128],
                                in1=dslice, op=mybir.AluOpType.mult)
                            nc.scalar.activation(
                                hT[:, :w * 128], g2f[:, :w * 128],
                                mybir.ActivationFunctionType.Identity,
                                bias=bias_sb[:, 5:6], scale=1.0)
                        for k in range(w):
                            b = blk0 + gi * 4 + k
                            hTb = hT[:, k * 128:(k + 1) * 128]
                            if layer == 0:
                                # g1T halves
                                g1ps = pdensep.tile([128, 512], F32, tag="pd")
                                for h in range(2):
                                    nc.tensor.matmul(
                                        out=g1ps[:, h * 128:(h + 1) * 128],
                                        lhsT=wg1_sb[:, h * 128:(h + 1) * 128],
                                        rhs=hTb, start=True, stop=True)
                                g1b = smp.tile([128, 256], BF16, tag="g1b")
                                for h in range(2):
                                    nc.scalar.activation(
                                        g1b[:, h * 128:(h + 1) * 128],
                                        g1ps[:, h * 128:(h + 1) * 128],
                                        mybir.ActivationFunctionType.Identity,
                                        bias=bias_sb[:, 3 + h:4 + h], scale=1.0)
                                for h in range(2):
                                    nc.sync.dma_start(
                                        out=g1T_d[h][:, b * 128:(b + 1) * 128],
                                        in_=g1b[:, h * 128:(h + 1) * 128])
                                # z block (node-major): lhsT = g1T half, rhs = Wg2 half
                                zps = pdensep.tile([128, 512], F32, tag="pd")
                                for h in range(2):
                                    nc.tensor.matmul(
                                        out=zps[:, :128],
                                        lhsT=g1b[:, h * 128:(h + 1) * 128],
                                        rhs=wg2_sb[h][:],
                                        start=(h == 0), stop=(h == 1))
                                zb = smp.tile([128, 128], BF16, tag="zb")
                                nc.vector.tensor_scalar_mul(
                                    out=zb[:], in0=zps[:, :128],
                                    scalar1=dinvc_sb[:, b:b + 1])
                                nc.sync.dma_start(
                                    out=zsh[b * 128:(b + 1) * 128, :], in_=zb[:])
                                # z block transposed (feature-major) for the
                                # layer-2 self-loop psum add
                                zTps = pdensep.tile([128, 512], F32, tag="pd")
                                for h in range(2):
                                    nc.tensor.matmul(
                                        out=zTps[:, :128],
                                        lhsT=wg2_sb[h][:],
                                        rhs=g1b[:, h * 128:(h + 1) * 128],
                                        start=(h == 0), stop=(h == 1))
                                zTb = smp.tile([128, 128], BF16, tag="zTb")
                                nc.vector.tensor_tensor(
                                    out=zTb[:], in0=zTps[:, :128],
                                    in1=dv[:, (gi * 4 + k) * 128:(gi * 4 + k + 1) * 128],
                                    op=mybir.AluOpType.mult)
                                nc.sync.dma_start(
                                    out=zTf_d[:, b * 128:(b + 1) * 128], in_=zTb[:])
                            else:
                                # x2 = W_out^T @ [xn; g1; g2]
                                xnb = smp.tile([128, 128], BF16, tag="xnb")
                                nc.scalar.dma_start(
                                    out=xnb[:], in_=xnT[:, b * 128:(b + 1) * 128])
                                g1b0 = smp.tile([128, 128], BF16, tag="g1r0")
                                g1b1 = smp.tile([128, 128], BF16, tag="g1r1")
                                nc.scalar.dma_start(
                                    out=g1b0[:], in_=g1T_d[0][:, b * 128:(b + 1) * 128])
                                nc.scalar.dma_start(
                                    out=g1b1[:], in_=g1T_d[1][:, b * 128:(b + 1) * 128])
                                xps = pdensep.tile([128, 512], F32, tag="pd")
                                nc.tensor.matmul(out=xps[:, :128], lhsT=wout_sb[0][:],
                                                 rhs=xnb[:], start=True, stop=False)
                                nc.tensor.matmul(out=xps[:, :128], lhsT=wout_sb[1][:],
                                                 rhs=g1b0[:], start=False, stop=False)
                                nc.tensor.matmul(out=xps[:, :128], lhsT=wout_sb[2][:],
                                                 rhs=g1b1[:], start=False, stop=False)
                                nc.tensor.matmul(out=xps[:, :128], lhsT=wout_sb[3][:],
                                                 rhs=hTb, start=False, stop=True)
                                x2b = outsp.tile([128, 128], F32, tag="x2b")
                                nc.scalar.activation(
                                    x2b[:], xps[:, :128],
                                    mybir.ActivationFunctionType.Identity,
                                    bias=bias_sb[:, 6:7], scale=1.0)
                                nc.sync.dma_start(
                                    out=x2_out[:, b * 128:(b + 1) * 128], in_=x2b[:])

            # dstl resident
            dstl_sb = constp.tile([128, sched["col_count"]], BF16, tag="dstl")
            nc.sync.dma_start(out=dstl_sb[:], in_=dstl_d[:, :])

            # per-queue DMA completion semaphores for prepare_only gathers
            dma_sems = [nc.alloc_semaphore(f"gsem{q}") for q in range(cfg.gq)]

            agg_layer(utable, uTf, 0)

            # AllGather z table
            nc.gpsimd.collective_compute(
                "AllGather", mybir.AluOpType.bypass,
                ins=[zsh.opt()],
                outs=[ztable[:cfg.trows, :].opt()],
                replica_groups=[list(range(cfg.ncores))],
            )

            agg_layer(ztable, zTf_d, 1)

    nc.compile()
    return nc


def make_inmaps(cfg, sched, inputs):
    K, PPC = cfg.ncores, cfg.ppc
    x_self = np.asarray(inputs["x_self"], np.float32)
    x_nb = np.asarray(inputs["x_neighbor"], np.float32)
    dinv = sched["dinv"]
    core_of, pos_of = sched["core_of"], sched["pos_of"]

    bf = ml_dtypes.bfloat16
    w = {k: np.asarray(inputs[k], np.float32) for k in
         ("W_in_self", "W_out_self", "Wg1", "Wg2", "W_out")}
    biases = np.zeros((128, 7), np.float32)
    biases[:, 0] = inputs["b_in_self"][:128]
    biases[:, 1] = inputs["b_in_self"][128:]
    biases[:, 2] = inputs["b_out_self"]
    biases[:, 3] = inputs["bg1"][:128]
    biases[:, 4] = inputs["bg1"][128:]
    biases[:, 5] = inputs["bg2"]
    biases[:, 6] = inputs["b_out"]

    # full replicated u table (bf16), rows laid out core-major
    ut_full = np.zeros((cfg.trows_pad if hasattr(cfg, 'trows_pad') else 0, 128),
                       np.float32)
    row_of = sched["core_of"] * PPC + sched["pos_of"]
    ut_full[row_of] = x_nb * dinv[:, None]
    ut_full = ut_full.astype(bf)

    in_maps = []
    for c in range(K):
        sel = core_of == c
        nodes = np.where(sel)[0]
        pos = pos_of[sel]
        xnT = np.zeros((128, PPC), np.float32)
        xnT[:, pos] = x_nb[nodes].T
        xsT = np.zeros((128, PPC), np.float32)
        xsT[:, pos] = x_self[nodes].T
        dv = np.zeros(PPC, np.float32)
        dv[pos] = dinv[nodes]
        dinvb = np.broadcast_to(dv[None, :], (128, PPC)).copy()
        dinvc = dv.reshape(cfg.nb, 128).T.copy()
        uTf_c = np.zeros((128, PPC), np.float32)
        uTf_c[:, pos] = (x_nb[nodes] * dinv[nodes, None]).T
        in_maps.append({
            "utable": ut_full,
            "uTf": uTf_c.astype(bf),
            "xnT": xnT.astype(bf),
            "xsT": xsT.astype(bf),
            "dinvb": dinvb,
            "dinvc": dinvc,
            "idx": sched["idx_arr"][c],
            "dstl": sched["dstl_cols"][c].astype(bf),
            "W_in_self": w["W_in_self"].astype(bf),
            "W_out_self": w["W_out_self"].astype(bf),
            "Wg1": w["Wg1"].astype(bf),
            "Wg2": w["Wg2"].astype(bf),
            "W_out": w["W_out"].astype(bf),
            "biases": biases,
        })
    return in_maps


def unshard(cfg, sched, results):
    N = cfg.N
    l1 = np.zeros((N, 128), np.float32)
    x2 = np.zeros((N, 128), np.float32)
    core_of, pos_of = sched["core_of"], sched["pos_of"]
    for c in range(cfg.ncores):
        sel = core_of == c
        nodes = np.where(sel)[0]
        pos = pos_of[sel]
        l1[nodes] = results[c]["l1T"].T[pos]
        x2[nodes] = results[c]["x2T"].T[pos]
    return l1, x2


def kernel(**inputs):
    cfg = CFG(N=inputs["x_self"].shape[0])
    sched = build_schedule(np.asarray(inputs["edge_index"]), cfg)
    nc = build_program(cfg, sched)
    in_maps = make_inmaps(cfg, sched, inputs)
    res = bass_utils.run_bass_kernel_spmd(
        nc, in_maps, core_ids=list(range(cfg.ncores)))
    l1, x2 = unshard(cfg, sched, res.results)
    return (l1, x2)



# revision 35
# speedup vs baseline: 1.2393x; 1.2393x over previous
"""Trainium2 Bass kernel for nn_LinearEncoder (2-layer GCN + dense branch).

Strategy (8 NeuronCores, SPMD):
  - Nodes are degree-sorted and dealt round-robin to 8 cores (load balance);
    each core owns PPC=12544 destination positions (98 blocks of 128).
  - GCN linearity: aggregate the 128-wide scaled node table u = x*dinv
    (resp. z = (g1@Wg2)*dinv for layer 2), then apply the weight matmul once.
  - Layer-1 table u is host-computed and replicated to every core as an
    input (no AllGather); the layer-2 z table is computed on-device and
    AllGather'd (bf16). Each core gathers its incoming-edge source rows
    with batched dma_gather (4096-idx instructions, int16 chunk-relative
    indices, 4 SWDGE queues) and segment-sums them on the TensorEngine via
    one-hot matmuls accumulated in PSUM (feat-major), superblock by
    superblock. Self-loops skip the gather: they are added straight into
    PSUM by identity-matrix matmuls over the local (dinv-scaled) table.
  - Dense branches are feat-major bf16 matmuls with biases/ReLU on the
    Scalar (ACT) engine; outputs are written transposed per core and
    un-permuted on the host.
"""

import numpy as np
import ml_dtypes

import concourse.bacc as bacc
import concourse.mybir as mybir
import concourse.tile as tile
from concourse import bass_utils

F32 = mybir.dt.float32
BF16 = mybir.dt.bfloat16
I16 = mybir.dt.int16
I32 = mybir.dt.int32
NEG = -1.0  # dstl mask value


class CFG:
    def __init__(self, N, ncores=8, ch_rows=32768, nidx=4096, sb_blocks=20):
        self.N = N
        self.ncores = ncores
        per = -(-N // ncores)
        self.per = per                      # real nodes per core (first cores)
        self.ppc = -(-per // 128) * 128     # padded per core
        self.nb = self.ppc // 128           # blocks per core
        self.trows = ncores * self.ppc      # real table rows
        self.ch = ch_rows
        self.nchunk = -(-self.trows // ch_rows)
        self.trows_pad = self.nchunk * ch_rows
        self.nidx = nidx
        self.gq = 4  # gather SWDGE queues (set 1 for CoreSim validation)
        # superblock partition of blocks
        sbs = []
        b = self.nb
        while b > 0:
            sbs.append(min(sb_blocks, b))
            b -= min(sb_blocks, b)
        self.sbs = sbs


def _deal_nodes(deg, cfg):
    """Degree-sorted round-robin deal of nodes to (core, pos)."""
    N = cfg.N
    order = np.argsort(-deg, kind="stable")
    core_of = np.empty(N, np.int64)
    pos_of = np.empty(N, np.int64)
    r = np.arange(N, dtype=np.int64)
    core_of[order] = r % cfg.ncores
    pos_of[order] = r // cfg.ncores
    return core_of, pos_of


def build_schedule(edge_index, cfg):
    """Static SPMD schedule + per-core device arrays, from the actual graph."""
    N, K = cfg.N, cfg.ncores
    src = np.asarray(edge_index[0], dtype=np.int64)
    dst = np.asarray(edge_index[1], dtype=np.int64)
    deg = np.bincount(dst, minlength=N).astype(np.int64) + 1
    dinv = (1.0 / np.sqrt(deg.astype(np.float64))).astype(np.float32)

    core_of, pos_of = _deal_nodes(deg, cfg)
    row_of = core_of * cfg.ppc + pos_of  # table row of each node

    # per-core real node counts
    npercore = np.bincount(core_of, minlength=K)

    # real edges only; self-loops are added on-device via identity matmuls
    esrc = src
    edst = dst

    e_core = core_of[edst]
    e_pos = pos_of[edst]
    e_blk = e_pos >> 7
    e_p = (e_pos & 127).astype(np.float32)
    e_rowsrc = row_of[esrc]
    e_ch = e_rowsrc // cfg.ch
    e_rel = (e_rowsrc % cfg.ch).astype(np.int16)

    nb, nch = cfg.nb, cfg.nchunk
    sb_of_blk = np.repeat(np.arange(len(cfg.sbs)), cfg.sbs)

    # group = (sb, ch, blk); order edges by (core, sb, ch, blk)
    g_of_e = (sb_of_blk[e_blk] * nch + e_ch) * nb + e_blk  # group id within core
    ngrp_ids = len(cfg.sbs) * nch * nb  # sparse (blk implies sb) but fine
    key = e_core * ngrp_ids + g_of_e
    eord = np.argsort(key, kind="stable")
    key_s = key[eord]

    # counts per (core, group)
    cnt = np.bincount(key_s, minlength=K * ngrp_ids).reshape(K, ngrp_ids)

    # group list in slot order: for sb, for ch, for blk in sb
    grp_list = []  # (sb, ch, blk, gid)
    for sbi, sbn in enumerate(cfg.sbs):
        blk0 = sum(cfg.sbs[:sbi])
        for ch in range(nch):
            for blk in range(blk0, blk0 + sbn):
                gid = (sbi * nch + ch) * nb + blk
                grp_list.append((sbi, ch, blk, gid))

    # padded group sizes: max over cores; chunk-0 groups at least 1
    gmax = {}
    for sbi, ch, blk, gid in grp_list:
        m = int(cnt[:, gid].max())
        if ch == 0:
            m = max(m, 1)
        gmax[gid] = m

    # per-(sb,ch) runs: pad total to x16; compute slot offsets
    runs = []  # (sbi, ch, slot_off, n_slots, [(blk, gid, off_in_run, gsize)])
    slot_blk_parts = []
    total = 0
    for sbi, sbn in enumerate(cfg.sbs):
        blk0 = sum(cfg.sbs[:sbi])
        for ch in range(nch):
            glist = []
            off = 0
            for blk in range(blk0, blk0 + sbn):
                gid = (sbi * nch + ch) * nb + blk
                gs = gmax[gid]
                if gs:
                    glist.append((blk, gid, off, gs))
                off += gs
            pad_tail = (-off) % 16
            n = off + pad_tail
            sb_slot_blk = np.full(n, -1, np.int64)
            for blk, gid, o, gs in glist:
                sb_slot_blk[o:o + gs] = blk
            runs.append((sbi, ch, total, n, glist))
            slot_blk_parts.append(sb_slot_blk)
            total += n
    n_slots = total
    slot_blk = np.concatenate(slot_blk_parts) if slot_blk_parts else np.zeros(0, np.int64)

    # pad rows per chunk (zero rows of the table): first padded position of
    # some core inside each chunk's row range
    pad_row_rel = np.full(nch, -1, np.int64)
    for c in range(K):
        if npercore[c] < cfg.ppc:
            r0 = c * cfg.ppc + npercore[c]
            ch = r0 // cfg.ch
            if pad_row_rel[ch] < 0:
                pad_row_rel[ch] = r0 % cfg.ch
    # fallback: fill missing chunks with any real zero... must not happen
    for ch in range(nch):
        if pad_row_rel[ch] < 0:
            # point at the last real row of the chunk; its value times a
            # zero one-hot column contributes nothing (dstl = -1 for pads)
            pad_row_rel[ch] = 0

    # per-core slot arrays: idx (int16 rel) + dstl (float p or -1)
    slot_idx = np.zeros((K, n_slots), np.int16)
    slot_dstl = np.full((K, n_slots), NEG, np.float32)
    # default pad idx per run
    for (sbi, ch, off, n, glist) in runs:
        slot_idx[:, off:off + n] = pad_row_rel[ch]
    # place real edges: rank within (core, group)
    grp_off = {}
    for (sbi, ch, off, n, glist) in runs:
        for blk, gid, o, gs in glist:
            grp_off[gid] = off + o
    # vectorized placement
    uk, inv = np.unique(key_s, return_inverse=True)
    starts = np.searchsorted(key_s, uk)
    rank = np.arange(len(key_s)) - starts[inv]
    core_s = key_s // ngrp_ids
    gid_s = key_s % ngrp_ids
    base = np.array([grp_off.get(int(g), -1) for g in uk % ngrp_ids], np.int64)
    slot_pos = base[inv] + rank
    assert (base[inv] >= 0).all()
    slot_idx[core_s, slot_pos] = e_rel[eord]
    slot_dstl[core_s, slot_pos] = e_p[eord]

    # instructions: slice each run into <= nidx pieces
    instrs = []  # (sbi, ch, idx_col_off, n_idx, units)
    # units: (g_tile, grp4, blk, col, start, stop)
    # dstl columns are appended as discovered
    col_count = 0
    unit_cols = []  # (slot_off_of_tile, n_valid, blk) for building dstl cols
    first_seen = {}
    last_seen = {}
    order_units = []
    for (sbi, ch, roff, rn, glist) in runs:
        o = 0
        while o < rn:
            n = min(cfg.nidx, rn - o)
            base_slot = roff + o
            units = []
            ntiles = -(-n // 128)
            for g in range(ntiles):
                t0 = base_slot + g * 128
                t1 = min(t0 + 128, base_slot + n)
                blks = np.unique(slot_blk[t0:t1])
                for blk in blks:
                    if blk < 0:
                        continue
                    col = col_count
                    col_count += 1
                    unit_cols.append((t0, t1 - t0, int(blk)))
                    u = [g, int(blk), col]
                    units.append(u)
                    kkey = (sbi, int(blk))
                    if kkey not in first_seen:
                        first_seen[kkey] = (len(instrs), len(units) - 1)
                    last_seen[kkey] = (len(instrs), len(units) - 1)
            instrs.append([sbi, ch, base_slot, n, units])
            o += n
    # mark start/stop
    for ii, (sbi, ch, base_slot, n, units) in enumerate(instrs):
        for ui, (g, blk, col) in enumerate(units):
            st = first_seen[(sbi, blk)] == (ii, ui)
            sp = last_seen[(sbi, blk)] == (ii, ui)
            units[ui] = (g, blk, col, st, sp)

    # per-core dstl column array [128, col_count]
    dstl_cols = np.full((K, 128, col_count), NEG, np.float32)
    for col, (t0, nvalid, blk) in enumerate(unit_cols):
        seg = slot_dstl[:, t0:t0 + nvalid]           # [K, nvalid]
        segblk = slot_blk[t0:t0 + nvalid]            # [nvalid]
        m = segblk == blk
        v = np.where(m[None, :], seg, NEG)
        dstl_cols[:, :nvalid, col] = v

    # per-core idx array wrapped per instruction: [128, n_slots/16]
    assert n_slots % 16 == 0
    idx_arr = np.zeros((K, 128, n_slots // 16), np.int16)
    for (sbi, ch, base_slot, n, units) in instrs:
        seg = slot_idx[:, base_slot:base_slot + n]    # [K, n]
        assert n % 16 == 0
        w = seg.reshape(K, n // 16, 16).transpose(0, 2, 1)  # [K,16,n/16]
        c0 = base_slot // 16
        for grp in range(8):
            idx_arr[:, 16 * grp:16 * grp + 16, c0:c0 + n // 16] = w

    return dict(
        deg=deg, dinv=dinv, core_of=core_of, pos_of=pos_of, row_of=row_of,
        npercore=npercore, instrs=instrs, n_slots=n_slots, col_count=col_count,
        idx_arr=idx_arr, dstl_cols=dstl_cols, runs=runs,
    )


def build_program(cfg, sched):
    nc = bacc.Bacc("TRN2", target_bir_lowering=False, debug=False,
                   num_devices=cfg.ncores, num_swdge_queues=cfg.gq)
    PPC, NB, CH, NCH = cfg.ppc, cfg.nb, cfg.ch, cfg.nchunk
    D = 128

    # ---- inputs ----
    utable = nc.dram_tensor("utable", [cfg.trows_pad, D], BF16, kind="ExternalInput")
    uTf = nc.dram_tensor("uTf", [D, PPC], BF16, kind="ExternalInput")
    xnT = nc.dram_tensor("xnT", [D, PPC], BF16, kind="ExternalInput")
    xsT = nc.dram_tensor("xsT", [D, PPC], BF16, kind="ExternalInput")
    dinvb = nc.dram_tensor("dinvb", [D, PPC], F32, kind="ExternalInput")
    dinvc = nc.dram_tensor("dinvc", [D, NB], F32, kind="ExternalInput")
    idx_d = nc.dram_tensor("idx", [D, sched["n_slots"] // 16], I16, kind="ExternalInput")
    dstl_d = nc.dram_tensor("dstl", [D, sched["col_count"]], BF16, kind="ExternalInput")
    w_ins = nc.dram_tensor("W_in_self", [D, 256], BF16, kind="ExternalInput")
    w_os = nc.dram_tensor("W_out_self", [384, D], BF16, kind="ExternalInput")
    wg1 = nc.dram_tensor("Wg1", [D, 256], BF16, kind="ExternalInput")
    wg2 = nc.dram_tensor("Wg2", [256, D], BF16, kind="ExternalInput")
    w_out = nc.dram_tensor("W_out", [512, D], BF16, kind="ExternalInput")
    biases = nc.dram_tensor("biases", [D, 7], F32, kind="ExternalInput")
    # bias cols: 0,1 b_in_self | 2 b_out_self | 3,4 bg1 | 5 bg2 | 6 b_out
    l1_out = nc.dram_tensor("l1T", [D, PPC], F32, kind="ExternalOutput")
    x2_out = nc.dram_tensor("x2T", [D, PPC], F32, kind="ExternalOutput")

    instrs = sched["instrs"]
    MAXU = max(len(u[4]) for u in instrs)

    with tile.TileContext(nc) as tc:
        with tc.tile_pool(name="const", bufs=1) as constp, \
             tc.tile_pool(name="dram", bufs=1, space="DRAM") as dramp, \
             tc.tile_pool(name="idxs", bufs=4) as idxsp, \
             tc.tile_pool(name="stage", bufs=8) as stagep, \
             tc.tile_pool(name="oh", bufs=3) as ohp, \
             tc.tile_pool(name="pagg", bufs=5, space="PSUM") as paggp, \
             tc.tile_pool(name="pdense", bufs=3, space="PSUM") as pdensep, \
             tc.tile_pool(name="hT", bufs=4) as hTp, \
             tc.tile_pool(name="sm", bufs=8) as smp, \
             tc.tile_pool(name="outs", bufs=4) as outsp, \
             tc.tile_pool(name="dinvs", bufs=2) as dinvsp:

            # constants
            iota_i32 = constp.tile([128, 128], I32)
            nc.gpsimd.iota(iota_i32[:], pattern=[[1, 128]], base=0, channel_multiplier=0)
            iota_bf = constp.tile([128, 128], BF16)
            nc.vector.tensor_copy(out=iota_bf[:], in_=iota_i32[:])
            zeros512 = constp.tile([128, 512], BF16)
            nc.vector.memset(zeros512[:], 0.0)
            # identity matrix for self-loop psum adds
            iota_ch = constp.tile([128, 128], I32)
            nc.gpsimd.iota(iota_ch[:], pattern=[[0, 128]], base=0, channel_multiplier=1)
            ident = constp.tile([128, 128], BF16)
            nc.vector.tensor_tensor(out=ident[:], in0=iota_i32[:], in1=iota_ch[:],
                                    op=mybir.AluOpType.is_equal)

            wins_sb = constp.tile([128, 256], BF16)
            nc.sync.dma_start(out=wins_sb[:], in_=w_ins[:, :])
            wos_sb = [constp.tile([128, 128], BF16, tag=f"wos{k}", name=f"wos{k}") for k in range(3)]
            for k in range(3):
                nc.sync.dma_start(out=wos_sb[k][:], in_=w_os[k * 128:(k + 1) * 128, :])
            wg1_sb = constp.tile([128, 256], BF16)
            nc.sync.dma_start(out=wg1_sb[:], in_=wg1[:, :])
            wg2_sb = [constp.tile([128, 128], BF16, tag=f"wg2{k}", name=f"wg2{k}") for k in range(2)]
            for k in range(2):
                nc.sync.dma_start(out=wg2_sb[k][:], in_=wg2[k * 128:(k + 1) * 128, :])
            wout_sb = [constp.tile([128, 128], BF16, tag=f"wo{k}", name=f"wo{k}") for k in range(4)]
            for k in range(4):
                nc.sync.dma_start(out=wout_sb[k][:], in_=w_out[k * 128:(k + 1) * 128, :])
            bias_sb = constp.tile([128, 7], F32)
            nc.sync.dma_start(out=bias_sb[:], in_=biases[:, :])
            dinvc_sb = constp.tile([128, NB], F32)
            nc.sync.dma_start(out=dinvc_sb[:], in_=dinvc[:, :])


            # z-table plumbing (written during L1 phase, AG'd before L2)
            zsh = dramp.tile([PPC, D], BF16)
            ztable = dramp.tile([cfg.trows_pad, D], BF16)
            zTf_d = dramp.tile([D, PPC], BF16)

            # g1T spill (two halves, feat-major)
            g1T_d = [dramp.tile([D, PPC], BF16, name=f"g1T{h}") for h in range(2)]

            # ---------- dense self branch (independent) ----------
            for b in range(NB):
                xs_blk = smp.tile([128, 128], BF16, tag="xs")
                nc.sync.dma_start(out=xs_blk[:], in_=xsT[:, b * 128:(b + 1) * 128])
                l1ps = pdensep.tile([128, 512], F32, tag="pd")
                for h in range(2):
                    nc.tensor.matmul(
                        out=l1ps[:, h * 128:(h + 1) * 128],
                        lhsT=wins_sb[:, h * 128:(h + 1) * 128],
                        rhs=xs_blk[:], start=True, stop=True)
                l1b = smp.tile([128, 256], BF16, tag="l1b")
                for h in range(2):
                    nc.scalar.activation(
                        l1b[:, h * 128:(h + 1) * 128],
                        l1ps[:, h * 128:(h + 1) * 128],
                        mybir.ActivationFunctionType.Relu,
                        bias=bias_sb[:, h:h + 1], scale=1.0)
                o1ps = pdensep.tile([128, 512], F32, tag="pd")
                nc.tensor.matmul(out=o1ps[:, :128], lhsT=wos_sb[0][:], rhs=xs_blk[:],
                                 start=True, stop=False)
                for h in range(2):
                    nc.tensor.matmul(out=o1ps[:, :128], lhsT=wos_sb[1 + h][:],
                                     rhs=l1b[:, h * 128:(h + 1) * 128],
                                     start=False, stop=(h == 1))
                o1 = outsp.tile([128, 128], F32, tag="o1")
                nc.scalar.activation(
                    o1[:], o1ps[:, :128],
                    mybir.ActivationFunctionType.Identity,
                    bias=bias_sb[:, 2:3], scale=1.0)
                nc.sync.dma_start(out=l1_out[:, b * 128:(b + 1) * 128], in_=o1[:])

            # ---------- per-layer aggregation ----------
            def agg_layer(table, selfT_d, layer):
                """Returns nothing; layer==0 computes g1T+z, layer==1 computes g2T+x2."""
                qn = [0]
                ii = 0
                n_instr = len(instrs)
                while ii < n_instr:
                    sbi = instrs[ii][0]
                    blk0 = sum(cfg.sbs[:sbi])
                    sbn = cfg.sbs[sbi]
                    ngrp = -(-sbn // 4)
                    # psum group tiles for this superblock
                    gtiles = [paggp.tile([128, 512], F32, tag="agg", name=f"agg_{layer}_{sbi}_{gg}") for gg in range(ngrp)]
                    for gt in gtiles:
                        nc.tensor.matmul(out=gt[:], lhsT=iota_bf[:], rhs=zeros512[:],
                                         start=True, stop=False)
                    # dinvb slice for this superblock
                    dv = dinvsp.tile([128, sbn * 128], F32, tag="dv")
                    nc.sync.dma_start(
                        out=dv[:], in_=dinvb[:, blk0 * 128:(blk0 + sbn) * 128])
                    # self-loop contributions (identity matmul into psum)
                    sfT = dinvsp.tile([128, sbn * 128], BF16, tag="sfT")
                    nc.scalar.dma_start(
                        out=sfT[:], in_=selfT_d[:, blk0 * 128:(blk0 + sbn) * 128])
                    for gi in range(ngrp):
                        w = min(4, sbn - gi * 4)
                        nc.tensor.matmul(
                            out=gtiles[gi][:, :w * 128], lhsT=ident[:],
                            rhs=sfT[:, gi * 512:gi * 512 + w * 128],
                            start=False, stop=False, skip_group_check=True)
                    # all instructions of this superblock (4 chunks)
                    while ii < n_instr and instrs[ii][0] == sbi:
                        _, ch, base_slot, n, units = instrs[ii]
                        idx_sb_t = idxsp.tile([128, -(-cfg.nidx // 16)], I16, tag="idx")
                        ncols16 = n // 16
                        nc.sync.dma_start(
                            out=idx_sb_t[:, :ncols16],
                            in_=idx_d[:, base_slot // 16: base_slot // 16 + ncols16])
                        stage = stagep.tile([128, cfg.nidx], BF16, tag="stage")
                        nc.gpsimd.dma_gather(
                            out_ap=stage[:, :(-(-n // 128)) * 128].rearrange(
                                "p (g e) -> p g e", e=D),
                            in_ap=table[ch * CH:(ch + 1) * CH, :],
                            idxs_ap=idx_sb_t[:, :ncols16],
                            num_idxs=n, num_idxs_reg=n, elem_size=D,
                            single_packet=False, queue_num=qn[0] % cfg.gq)
                        qn[0] += 1
                        nu = len(units)
                        oh = ohp.tile([128, MAXU * 128], BF16, tag="oh")
                        c0 = units[0][2]
                        assert units[-1][2] - c0 + 1 == nu
                        nc.vector.tensor_tensor(
                            out=oh[:, :nu * 128].rearrange("p (u e) -> p u e", e=128),
                            in0=iota_bf[:].rearrange("p (a e) -> p a e", a=1)
                                .to_broadcast([128, nu, 128]),
                            in1=dstl_sb[:, c0:c0 + nu].to_broadcast([128, nu, 128]),
                            op=mybir.AluOpType.is_equal)
                        stage3 = stage[:].rearrange("p (g e) -> p g e", e=D)
                        for (g, blk, col, st, sp) in units:
                            gi = (blk - blk0) // 4
                            sl = (blk - blk0) % 4
                            kk = min(128, n - g * 128)
                            nc.tensor.matmul(
                                out=gtiles[gi][:, sl * 128:(sl + 1) * 128],
                                lhsT=stage3[:kk, g, :],
                                rhs=oh[:kk, (col - c0) * 128:(col - c0 + 1) * 128],
                                start=False, stop=False, skip_group_check=True)
                        ii += 1
                    for gt in gtiles:
                        nc.tensor.matmul(out=gt[:], lhsT=iota_bf[:], rhs=zeros512[:],
                                         start=False, stop=True)
                    # final pass per group + per-block dense work
                    for gi in range(ngrp):
                        w = min(4, sbn - gi * 4)
                        hT = hTp.tile([128, 512], BF16, tag="hT")
                        dslice = dv[:, gi * 4 * 128:(gi * 4 + w) * 128]
                        if layer == 0:
                            nc.vector.tensor_tensor(
                                out=hT[:, :w * 128], in0=gtiles[gi][:, :w * 128],
                                in1=dslice, op=mybir.AluOpType.mult)
                        else:
                            g2f = hTp.tile([128, 512], F32, tag="g2f")
                            nc.vector.tensor_tensor(
                                out=g2f[:, :w * 128], in0=gtiles[gi][:, :w * 128],
                                in1=dslice, op=mybir.AluOpType.mult)
                            nc.scalar.activation(
                                hT[:, :w * 128], g2f[:, :w * 128],
                                mybir.ActivationFunctionType.Identity,
                                bias=bias_sb[:, 5:6], scale=1.0)
                        for k in range(w):
                            b = blk0 + gi * 4 + k
                            hTb = hT[:, k * 128:(k + 1) * 128]
                            if layer == 0:
                                # g1T halves
                                g1ps = pdensep.tile([128, 512], F32, tag="pd")
                                for h in range(2):
                                    nc.tensor.matmul(
                                        out=g1ps[:, h * 128:(h + 1) * 128],
                                        lhsT=wg1_sb[:, h * 128:(h + 1) * 128],
                                        rhs=hTb, start=True, stop=True)
                                g1b = smp.tile([128, 256], BF16, tag="g1b")
                                for h in range(2):
                                    nc.scalar.activation(
                                        g1b[:, h * 128:(h + 1) * 128],
                                        g1ps[:, h * 128:(h + 1) * 128],
                                        mybir.ActivationFunctionType.Identity,
                                        bias=bias_sb[:, 3 + h:4 + h], scale=1.0)
                                for h in range(2):
                                    nc.sync.dma_start(
                                        out=g1T_d[h][:, b * 128:(b + 1) * 128],
                                        in_=g1b[:, h * 128:(h + 1) * 128])
                                # z block (node-major): lhsT = g1T half, rhs = Wg2 half
                                zps = pdensep.tile([128, 512], F32, tag="pd")
                                for h in range(2):
                                    nc.tensor.matmul(
                                        out=zps[:, :128],
                                        lhsT=g1b[:, h * 128:(h + 1) * 128],
                                        rhs=wg2_sb[h][:],
                                        start=(h == 0), stop=(h == 1))
                                zb = smp.tile([128, 128], BF16, tag="zb")
                                nc.vector.tensor_scalar_mul(
                                    out=zb[:], in0=zps[:, :128],
                                    scalar1=dinvc_sb[:, b:b + 1])
                                nc.sync.dma_start(
                                    out=zsh[b * 128:(b + 1) * 128, :], in_=zb[:])
                                # z block transposed (feature-major) for the
                                # layer-2 self-loop psum add
                                zTps = pdensep.tile([128, 512], F32, tag="pd")
                                for h in range(2):
                                    nc.tensor.matmul(
                                        out=zTps[:, :128],
                                        lhsT=wg2_sb[h][:],
                                        rhs=g1b[:, h * 128:(h + 1) * 128],
                                        start=(h == 0), stop=(h == 1))
                                zTb = smp.tile([128, 128], BF16, tag="zTb")
                                nc.vector.tensor_tensor(
                                    out=zTb[:], in0=zTps[:, :128],
                                    in1=dv[:, (gi * 4 + k) * 128:(gi * 4 + k + 1) * 128],
                                    op=mybir.AluOpType.mult)
                                nc.sync.dma_start(
                                    out=zTf_d[:, b * 128:(b + 1) * 128], in_=zTb[:])
                            else:
                                # x2 = W_out^T @ [xn; g1; g2]
                                xnb = smp.tile([128, 128], BF16, tag="xnb")
                                nc.scalar.dma_start(
                                    out=xnb[:], in_=xnT[:, b * 128:(b + 1) * 128])
                                g1b0 = smp.tile([128, 128], BF16, tag="g1r0")
                                g1b1 = smp.tile([128, 128], BF16, tag="g1r1")
                                nc.scalar.dma_start(
                                    out=g1b0[:], in_=g1T_d[0][:, b * 128:(b + 1) * 128])
                                nc.scalar.dma_start(
                                    out=g1b1[:], in_=g1T_d[1][:, b * 128:(b + 1) * 128])
                                xps = pdensep.tile([128, 512], F32, tag="pd")
                                nc.tensor.matmul(out=xps[:, :128], lhsT=wout_sb[0][:],
                                                 rhs=xnb[:], start=True, stop=False)
                                nc.tensor.matmul(out=xps[:, :128], lhsT=wout_sb[1][:],
                                                 rhs=g1b0[:], start=False, stop=False)
                                nc.tensor.matmul(out=xps[:, :128], lhsT=wout_sb[2][:],
                                                 rhs=g1b1[:], start=False, stop=False)
                                nc.tensor.matmul(out=xps[:, :128], lhsT=wout_sb[3][:],
                                                 rhs=hTb, start=False, stop=True)
                                x2b = outsp.tile([128, 128], F32, tag="x2b")
                                nc.scalar.activation(
                                    x2b[:], xps[:, :128],
                                    mybir.ActivationFunctionType.Identity,
                                    bias=bias_sb[:, 6:7], scale=1.0)
                                nc.sync.dma_start(
                                    out=x2_out[:, b * 128:(b + 1) * 128], in_=x2b[:])

            # dstl resident
            dstl_sb = constp.tile([128, sched["col_count"]], BF16, tag="dstl")
            nc.sync.dma_start(out=dstl_sb[:], in_=dstl_d[:, :])

            # per-queue DMA completion semaphores for prepare_only gathers
            dma_sems = [nc.alloc_semaphore(f"gsem{q}") for q in range(cfg.gq)]

            agg_layer(utable, uTf, 0)

            # AllGather z table
            nc.gpsimd.collective_compute(
                "AllGather", mybir.AluOpType.bypass,
                ins=[zsh.opt()],
                outs=[ztable[:cfg.trows, :].opt()],
                replica_groups=[list(range(cfg.ncores))],
            )

            agg_layer(ztable, zTf_d, 1)

    nc.compile()
    return nc


def make_inmaps(cfg, sched, inputs):
    K, PPC = cfg.ncores, cfg.ppc
    x_self = np.asarray(inputs["x_self"], np.float32)
    x_nb = np.asarray(inputs["x_neighbor"], np.float32)
    dinv = sched["dinv"]
    core_of, pos_of = sched["core_of"], sched["pos_of"]

    bf = ml_dtypes.bfloat16
    w = {k: np.asarray(inputs[k], np.float32) for k in
         ("W_in_self", "W_out_self", "Wg1", "Wg2", "W_out")}
    biases = np.zeros((128, 7), np.float32)
    biases[:, 0] = inputs["b_in_self"][:128]
    biases[:, 1] = inputs["b_in_self"][128:]
    biases[:, 2] = inputs["b_out_self"]
    biases[:, 3] = inputs["bg1"][:128]
    biases[:, 4] = inputs["bg1"][128:]
    biases[:, 5] = inputs["bg2"]
    biases[:, 6] = inputs["b_out"]

    # full replicated u table (bf16), rows laid out core-major
    ut_full = np.zeros((cfg.trows_pad if hasattr(cfg, 'trows_pad') else 0, 128),
                       np.float32)
    row_of = sched["core_of"] * PPC + sched["pos_of"]
    ut_full[row_of] = x_nb * dinv[:, None]
    ut_full = ut_full.astype(bf)

    in_maps = []
    for c in range(K):
        sel = core_of == c
        nodes = np.where(sel)[0]
        pos = pos_of[sel]
        xnT = np.zeros((128, PPC), np.float32)
        xnT[:, pos] = x_nb[nodes].T
        xsT = np.zeros((128, PPC), np.float32)
        xsT[:, pos] = x_self[nodes].T
        dv = np.zeros(PPC, np.float32)
        dv[pos] = dinv[nodes]
        dinvb = np.broadcast_to(dv[None, :], (128, PPC)).copy()
        dinvc = dv.reshape(cfg.nb, 128).T.copy()
        uTf_c = np.zeros((128, PPC), np.float32)
        uTf_c[:, pos] = (x_nb[nodes] * dinv[nodes, None]).T
        in_maps.append({
            "utable": ut_full,
            "uTf": uTf_c.astype(bf),
            "xnT": xnT.astype(bf),
            "xsT": xsT.astype(bf),
            "dinvb": dinvb,
            "dinvc": dinvc,
            "idx": sched["idx_arr"][c],
            "dstl": sched["dstl_cols"][c].astype(bf),
            "W_in_self": w["W_in_self"].astype(bf),
            "W_out_self": w["W_out_self"].astype(bf),
            "Wg1": w["Wg1"].astype(bf),
            "Wg2": w["Wg2"].astype(bf),
            "W_out": w["W_out"].astype(bf),
            "biases": biases,
        })
    return in_maps


def unshard(cfg, sched, results):
    N = cfg.N
    l1 = np.zeros((N, 128), np.float32)
    x2 = np.zeros((N, 128), np.float32)
    core_of, pos_of = sched["core_of"], sched["pos_of"]
    for c in range(cfg.ncores):
        sel = core_of == c
        nodes = np.where(sel)[0]
        pos = pos_of[sel]
        l1[nodes] = results[c]["l1T"].T[pos]
        x2[nodes] = results[c]["x2T"].T[pos]
    return l1, x2


def kernel(**inputs):
    cfg = CFG(N=inputs["x_self"].shape[0])
    sched = build_schedule(np.asarray(inputs["edge_index"]), cfg)
    nc = build_program(cfg, sched)
    in_maps = make_inmaps(cfg, sched, inputs)
    res = bass_utils.run_bass_kernel_spmd(
        nc, in_maps, core_ids=list(range(cfg.ncores)))
    l1, x2 = unshard(cfg, sched, res.results)
    return (l1, x2)



# revision 36
# speedup vs baseline: 1.2656x; 1.0213x over previous
"""Trainium2 Bass kernel for nn_LinearEncoder (2-layer GCN + dense branch).

Strategy (8 NeuronCores, SPMD):
  - Nodes are degree-sorted and dealt round-robin to 8 cores (load balance);
    each core owns PPC=12544 destination positions (98 blocks of 128).
  - GCN linearity: aggregate the 128-wide scaled node table u = x*dinv
    (resp. z = (g1@Wg2)*dinv for layer 2), then apply the weight matmul once.
  - Layer-1 table u is host-computed and replicated to every core as an
    input (no AllGather); the layer-2 z table is computed on-device and
    AllGather'd (bf16). Each core gathers its incoming-edge source rows
    with batched dma_gather (4096-idx instructions, int16 chunk-relative
    indices, 4 SWDGE queues) and segment-sums them on the TensorEngine via
    one-hot matmuls accumulated in PSUM (feat-major), superblock by
    superblock. Self-loops skip the gather: they are added straight into
    PSUM by identity-matrix matmuls over the local (dinv-scaled) table.
  - Dense branches are feat-major bf16 matmuls with biases/ReLU on the
    Scalar (ACT) engine; outputs are written transposed per core and
    un-permuted on the host.
"""

import numpy as np
import ml_dtypes

import concourse.bacc as bacc
import concourse.mybir as mybir
import concourse.tile as tile
from concourse import bass_utils

F32 = mybir.dt.float32
BF16 = mybir.dt.bfloat16
I16 = mybir.dt.int16
I32 = mybir.dt.int32
NEG = -1.0  # dstl mask value


class CFG:
    def __init__(self, N, ncores=8, ch_rows=32768, nidx=4096, sb_blocks=20):
        self.N = N
        self.ncores = ncores
        per = -(-N // ncores)
        self.per = per                      # real nodes per core (first cores)
        self.ppc = -(-per // 128) * 128     # padded per core
        self.nb = self.ppc // 128           # blocks per core
        self.trows = ncores * self.ppc      # real table rows
        self.ch = ch_rows
        self.nchunk = -(-self.trows // ch_rows)
        self.trows_pad = self.nchunk * ch_rows
        self.nidx = nidx
        self.gq = 4  # gather SWDGE queues (set 1 for CoreSim validation)
        # superblock partition of blocks
        sbs = []
        b = self.nb
        while b > 0:
            sbs.append(min(sb_blocks, b))
            b -= min(sb_blocks, b)
        self.sbs = sbs


def _deal_nodes(deg, cfg):
    """Degree-sorted round-robin deal of nodes to (core, pos)."""
    N = cfg.N
    order = np.argsort(-deg, kind="stable")
    core_of = np.empty(N, np.int64)
    pos_of = np.empty(N, np.int64)
    r = np.arange(N, dtype=np.int64)
    core_of[order] = r % cfg.ncores
    pos_of[order] = r // cfg.ncores
    return core_of, pos_of


def build_schedule(edge_index, cfg):
    """Static SPMD schedule + per-core device arrays, from the actual graph."""
    N, K = cfg.N, cfg.ncores
    src = np.asarray(edge_index[0], dtype=np.int64)
    dst = np.asarray(edge_index[1], dtype=np.int64)
    deg = np.bincount(dst, minlength=N).astype(np.int64) + 1
    dinv = (1.0 / np.sqrt(deg.astype(np.float64))).astype(np.float32)

    core_of, pos_of = _deal_nodes(deg, cfg)
    row_of = core_of * cfg.ppc + pos_of  # table row of each node

    # per-core real node counts
    npercore = np.bincount(core_of, minlength=K)

    # real edges only; self-loops are added on-device via identity matmuls
    esrc = src
    edst = dst

    e_core = core_of[edst]
    e_pos = pos_of[edst]
    e_blk = e_pos >> 7
    e_p = (e_pos & 127).astype(np.float32)
    e_rowsrc = row_of[esrc]
    e_ch = e_rowsrc // cfg.ch
    e_rel = (e_rowsrc % cfg.ch).astype(np.int16)

    nb, nch = cfg.nb, cfg.nchunk
    sb_of_blk = np.repeat(np.arange(len(cfg.sbs)), cfg.sbs)

    # group = (sb, ch, blk); order edges by (core, sb, ch, blk)
    g_of_e = (sb_of_blk[e_blk] * nch + e_ch) * nb + e_blk  # group id within core
    ngrp_ids = len(cfg.sbs) * nch * nb  # sparse (blk implies sb) but fine
    key = e_core * ngrp_ids + g_of_e
    eord = np.argsort(key, kind="stable")
    key_s = key[eord]

    # counts per (core, group)
    cnt = np.bincount(key_s, minlength=K * ngrp_ids).reshape(K, ngrp_ids)

    # group list in slot order: for sb, for ch, for blk in sb
    grp_list = []  # (sb, ch, blk, gid)
    for sbi, sbn in enumerate(cfg.sbs):
        blk0 = sum(cfg.sbs[:sbi])
        for ch in range(nch):
            for blk in range(blk0, blk0 + sbn):
                gid = (sbi * nch + ch) * nb + blk
                grp_list.append((sbi, ch, blk, gid))

    # padded group sizes: max over cores; chunk-0 groups at least 1
    gmax = {}
    for sbi, ch, blk, gid in grp_list:
        m = int(cnt[:, gid].max())
        if ch == 0:
            m = max(m, 1)
        gmax[gid] = m

    # per-(sb,ch) runs: pad total to x16; compute slot offsets
    runs = []  # (sbi, ch, slot_off, n_slots, [(blk, gid, off_in_run, gsize)])
    slot_blk_parts = []
    total = 0
    for sbi, sbn in enumerate(cfg.sbs):
        blk0 = sum(cfg.sbs[:sbi])
        for ch in range(nch):
            glist = []
            off = 0
            for blk in range(blk0, blk0 + sbn):
                gid = (sbi * nch + ch) * nb + blk
                gs = gmax[gid]
                if gs:
                    glist.append((blk, gid, off, gs))
                off += gs
            pad_tail = (-off) % 16
            n = off + pad_tail
            sb_slot_blk = np.full(n, -1, np.int64)
            for blk, gid, o, gs in glist:
                sb_slot_blk[o:o + gs] = blk
            runs.append((sbi, ch, total, n, glist))
            slot_blk_parts.append(sb_slot_blk)
            total += n
    n_slots = total
    slot_blk = np.concatenate(slot_blk_parts) if slot_blk_parts else np.zeros(0, np.int64)

    # pad rows per chunk (zero rows of the table): first padded position of
    # some core inside each chunk's row range
    pad_row_rel = np.full(nch, -1, np.int64)
    for c in range(K):
        if npercore[c] < cfg.ppc:
            r0 = c * cfg.ppc + npercore[c]
            ch = r0 // cfg.ch
            if pad_row_rel[ch] < 0:
                pad_row_rel[ch] = r0 % cfg.ch
    # fallback: fill missing chunks with any real zero... must not happen
    for ch in range(nch):
        if pad_row_rel[ch] < 0:
            # point at the last real row of the chunk; its value times a
            # zero one-hot column contributes nothing (dstl = -1 for pads)
            pad_row_rel[ch] = 0

    # per-core slot arrays: idx (int16 rel) + dstl (float p or -1)
    slot_idx = np.zeros((K, n_slots), np.int16)
    slot_dstl = np.full((K, n_slots), NEG, np.float32)
    # default pad idx per run
    for (sbi, ch, off, n, glist) in runs:
        slot_idx[:, off:off + n] = pad_row_rel[ch]
    # place real edges: rank within (core, group)
    grp_off = {}
    for (sbi, ch, off, n, glist) in runs:
        for blk, gid, o, gs in glist:
            grp_off[gid] = off + o
    # vectorized placement
    uk, inv = np.unique(key_s, return_inverse=True)
    starts = np.searchsorted(key_s, uk)
    rank = np.arange(len(key_s)) - starts[inv]
    core_s = key_s // ngrp_ids
    gid_s = key_s % ngrp_ids
    base = np.array([grp_off.get(int(g), -1) for g in uk % ngrp_ids], np.int64)
    slot_pos = base[inv] + rank
    assert (base[inv] >= 0).all()
    slot_idx[core_s, slot_pos] = e_rel[eord]
    slot_dstl[core_s, slot_pos] = e_p[eord]

    # instructions: slice each run into <= nidx pieces
    instrs = []  # (sbi, ch, idx_col_off, n_idx, units)
    # units: (g_tile, grp4, blk, col, start, stop)
    # dstl columns are appended as discovered
    col_count = 0
    unit_cols = []  # (slot_off_of_tile, n_valid, blk) for building dstl cols
    first_seen = {}
    last_seen = {}
    order_units = []
    for (sbi, ch, roff, rn, glist) in runs:
        o = 0
        while o < rn:
            n = min(cfg.nidx, rn - o)
            base_slot = roff + o
            units = []
            ntiles = -(-n // 128)
            for g in range(ntiles):
                t0 = base_slot + g * 128
                t1 = min(t0 + 128, base_slot + n)
                blks = np.unique(slot_blk[t0:t1])
                for blk in blks:
                    if blk < 0:
                        continue
                    col = col_count
                    col_count += 1
                    unit_cols.append((t0, t1 - t0, int(blk)))
                    u = [g, int(blk), col]
                    units.append(u)
                    kkey = (sbi, int(blk))
                    if kkey not in first_seen:
                        first_seen[kkey] = (len(instrs), len(units) - 1)
                    last_seen[kkey] = (len(instrs), len(units) - 1)
            instrs.append([sbi, ch, base_slot, n, units])
            o += n
    # mark start/stop
    for ii, (sbi, ch, base_slot, n, units) in enumerate(instrs):
        for ui, (g, blk, col) in enumerate(units):
            st = first_seen[(sbi, blk)] == (ii, ui)
            sp = last_seen[(sbi, blk)] == (ii, ui)
            units[ui] = (g, blk, col, st, sp)

    # per-core dstl column array [128, col_count]
    dstl_cols = np.full((K, 128, col_count), NEG, np.float32)
    for col, (t0, nvalid, blk) in enumerate(unit_cols):
        seg = slot_dstl[:, t0:t0 + nvalid]           # [K, nvalid]
        segblk = slot_blk[t0:t0 + nvalid]            # [nvalid]
        m = segblk == blk
        v = np.where(m[None, :], seg, NEG)
        dstl_cols[:, :nvalid, col] = v

    # per-core idx array wrapped per instruction: [128, n_slots/16]
    assert n_slots % 16 == 0
    idx_arr = np.zeros((K, 128, n_slots // 16), np.int16)
    for (sbi, ch, base_slot, n, units) in instrs:
        seg = slot_idx[:, base_slot:base_slot + n]    # [K, n]
        assert n % 16 == 0
        w = seg.reshape(K, n // 16, 16).transpose(0, 2, 1)  # [K,16,n/16]
        c0 = base_slot // 16
        for grp in range(8):
            idx_arr[:, 16 * grp:16 * grp + 16, c0:c0 + n // 16] = w

    return dict(
        deg=deg, dinv=dinv, core_of=core_of, pos_of=pos_of, row_of=row_of,
        npercore=npercore, instrs=instrs, n_slots=n_slots, col_count=col_count,
        idx_arr=idx_arr, dstl_cols=dstl_cols, runs=runs,
    )


def build_program(cfg, sched):
    nc = bacc.Bacc("TRN2", target_bir_lowering=False, debug=False,
                   num_devices=cfg.ncores, num_swdge_queues=cfg.gq)
    PPC, NB, CH, NCH = cfg.ppc, cfg.nb, cfg.ch, cfg.nchunk
    D = 128

    # ---- inputs ----
    utable = nc.dram_tensor("utable", [cfg.trows_pad, D], BF16, kind="ExternalInput")
    uTf = nc.dram_tensor("uTf", [D, PPC], BF16, kind="ExternalInput")
    xnT = nc.dram_tensor("xnT", [D, PPC], BF16, kind="ExternalInput")
    xsT = nc.dram_tensor("xsT", [D, PPC], BF16, kind="ExternalInput")
    dinvb = nc.dram_tensor("dinvb", [D, PPC], F32, kind="ExternalInput")
    dinvc = nc.dram_tensor("dinvc", [D, NB], F32, kind="ExternalInput")
    idx_d = nc.dram_tensor("idx", [D, sched["n_slots"] // 16], I16, kind="ExternalInput")
    dstl_d = nc.dram_tensor("dstl", [D, sched["col_count"]], BF16, kind="ExternalInput")
    w_ins = nc.dram_tensor("W_in_self", [D, 256], BF16, kind="ExternalInput")
    w_os = nc.dram_tensor("W_out_self", [384, D], BF16, kind="ExternalInput")
    wg1 = nc.dram_tensor("Wg1", [D, 256], BF16, kind="ExternalInput")
    wg2 = nc.dram_tensor("Wg2", [256, D], BF16, kind="ExternalInput")
    w_out = nc.dram_tensor("W_out", [512, D], BF16, kind="ExternalInput")
    biases = nc.dram_tensor("biases", [D, 7], F32, kind="ExternalInput")
    # bias cols: 0,1 b_in_self | 2 b_out_self | 3,4 bg1 | 5 bg2 | 6 b_out
    l1_out = nc.dram_tensor("l1T", [D, PPC], F32, kind="ExternalOutput")
    x2_out = nc.dram_tensor("x2T", [D, PPC], F32, kind="ExternalOutput")

    instrs = sched["instrs"]
    MAXU = max(len(u[4]) for u in instrs)

    with tile.TileContext(nc) as tc:
        with tc.tile_pool(name="const", bufs=1) as constp, \
             tc.tile_pool(name="dram", bufs=1, space="DRAM") as dramp, \
             tc.tile_pool(name="idxs", bufs=4) as idxsp, \
             tc.tile_pool(name="stage", bufs=8) as stagep, \
             tc.tile_pool(name="oh", bufs=3) as ohp, \
             tc.tile_pool(name="pagg", bufs=5, space="PSUM") as paggp, \
             tc.tile_pool(name="pdense", bufs=3, space="PSUM") as pdensep, \
             tc.tile_pool(name="hT", bufs=4) as hTp, \
             tc.tile_pool(name="sm", bufs=8) as smp, \
             tc.tile_pool(name="outs", bufs=4) as outsp, \
             tc.tile_pool(name="dinvs", bufs=2) as dinvsp:

            # constants
            iota_i32 = constp.tile([128, 128], I32)
            nc.gpsimd.iota(iota_i32[:], pattern=[[1, 128]], base=0, channel_multiplier=0)
            iota_bf = constp.tile([128, 128], BF16)
            nc.vector.tensor_copy(out=iota_bf[:], in_=iota_i32[:])
            zeros512 = constp.tile([128, 512], BF16)
            nc.vector.memset(zeros512[:], 0.0)
            # identity matrix for self-loop psum adds
            iota_ch = constp.tile([128, 128], I32)
            nc.gpsimd.iota(iota_ch[:], pattern=[[0, 128]], base=0, channel_multiplier=1)
            ident = constp.tile([128, 128], BF16)
            nc.vector.tensor_tensor(out=ident[:], in0=iota_i32[:], in1=iota_ch[:],
                                    op=mybir.AluOpType.is_equal)

            wins_sb = constp.tile([128, 256], BF16)
            nc.sync.dma_start(out=wins_sb[:], in_=w_ins[:, :])
            wos_sb = [constp.tile([128, 128], BF16, tag=f"wos{k}", name=f"wos{k}") for k in range(3)]
            for k in range(3):
                nc.sync.dma_start(out=wos_sb[k][:], in_=w_os[k * 128:(k + 1) * 128, :])
            wg1_sb = constp.tile([128, 256], BF16)
            nc.sync.dma_start(out=wg1_sb[:], in_=wg1[:, :])
            wg2_sb = [constp.tile([128, 128], BF16, tag=f"wg2{k}", name=f"wg2{k}") for k in range(2)]
            for k in range(2):
                nc.sync.dma_start(out=wg2_sb[k][:], in_=wg2[k * 128:(k + 1) * 128, :])
            wout_sb = [constp.tile([128, 128], BF16, tag=f"wo{k}", name=f"wo{k}") for k in range(4)]
            for k in range(4):
                nc.sync.dma_start(out=wout_sb[k][:], in_=w_out[k * 128:(k + 1) * 128, :])
            bias_sb = constp.tile([128, 7], F32)
            nc.sync.dma_start(out=bias_sb[:], in_=biases[:, :])
            dinvc_sb = constp.tile([128, NB], F32)
            nc.sync.dma_start(out=dinvc_sb[:], in_=dinvc[:, :])


            # z-table plumbing (written during L1 phase, AG'd before L2)
            zsh = dramp.tile([PPC, D], BF16)
            ztable = dramp.tile([cfg.trows_pad, D], BF16)
            zTf_d = dramp.tile([D, PPC], BF16)

            # g1T spill (two halves, feat-major)
            g1T_d = [dramp.tile([D, PPC], BF16, name=f"g1T{h}") for h in range(2)]

            # ---------- dense self branch (independent) ----------
            for b in range(NB):
                xs_blk = smp.tile([128, 128], BF16, tag="xs")
                nc.scalar.dma_start(out=xs_blk[:], in_=xsT[:, b * 128:(b + 1) * 128])
                l1ps = pdensep.tile([128, 512], F32, tag="pd")
                for h in range(2):
                    nc.tensor.matmul(
                        out=l1ps[:, h * 128:(h + 1) * 128],
                        lhsT=wins_sb[:, h * 128:(h + 1) * 128],
                        rhs=xs_blk[:], start=True, stop=True)
                l1b = smp.tile([128, 256], BF16, tag="l1b")
                for h in range(2):
                    nc.scalar.activation(
                        l1b[:, h * 128:(h + 1) * 128],
                        l1ps[:, h * 128:(h + 1) * 128],
                        mybir.ActivationFunctionType.Relu,
                        bias=bias_sb[:, h:h + 1], scale=1.0)
                o1ps = pdensep.tile([128, 512], F32, tag="pd")
                nc.tensor.matmul(out=o1ps[:, :128], lhsT=wos_sb[0][:], rhs=xs_blk[:],
                                 start=True, stop=False)
                for h in range(2):
                    nc.tensor.matmul(out=o1ps[:, :128], lhsT=wos_sb[1 + h][:],
                                     rhs=l1b[:, h * 128:(h + 1) * 128],
                                     start=False, stop=(h == 1))
                o1 = outsp.tile([128, 128], F32, tag="o1")
                nc.scalar.activation(
                    o1[:], o1ps[:, :128],
                    mybir.ActivationFunctionType.Identity,
                    bias=bias_sb[:, 2:3], scale=1.0)
                nc.sync.dma_start(out=l1_out[:, b * 128:(b + 1) * 128], in_=o1[:])

            # ---------- per-layer aggregation ----------
            def agg_layer(table, selfT_d, layer):
                """Returns nothing; layer==0 computes g1T+z, layer==1 computes g2T+x2."""
                qn = [0]
                ii = 0
                n_instr = len(instrs)
                while ii < n_instr:
                    sbi = instrs[ii][0]
                    blk0 = sum(cfg.sbs[:sbi])
                    sbn = cfg.sbs[sbi]
                    ngrp = -(-sbn // 4)
                    # psum group tiles for this superblock
                    gtiles = [paggp.tile([128, 512], F32, tag="agg", name=f"agg_{layer}_{sbi}_{gg}") for gg in range(ngrp)]
                    for gt in gtiles:
                        nc.tensor.matmul(out=gt[:], lhsT=iota_bf[:], rhs=zeros512[:],
                                         start=True, stop=False)
                    # dinvb slice for this superblock
                    dv = dinvsp.tile([128, sbn * 128], F32, tag="dv")
                    nc.sync.dma_start(
                        out=dv[:], in_=dinvb[:, blk0 * 128:(blk0 + sbn) * 128])
                    # self-loop contributions (identity matmul into psum)
                    sfT = dinvsp.tile([128, sbn * 128], BF16, tag="sfT")
                    nc.sync.dma_start(
                        out=sfT[:], in_=selfT_d[:, blk0 * 128:(blk0 + sbn) * 128])
                    for gi in range(ngrp):
                        w = min(4, sbn - gi * 4)
                        nc.tensor.matmul(
                            out=gtiles[gi][:, :w * 128], lhsT=ident[:],
                            rhs=sfT[:, gi * 512:gi * 512 + w * 128],
                            start=False, stop=False, skip_group_check=True)
                    # all instructions of this superblock (4 chunks)
                    while ii < n_instr and instrs[ii][0] == sbi:
                        _, ch, base_slot, n, units = instrs[ii]
                        idx_sb_t = idxsp.tile([128, -(-cfg.nidx // 16)], I16, tag="idx")
                        ncols16 = n // 16
                        nc.sync.dma_start(
                            out=idx_sb_t[:, :ncols16],
                            in_=idx_d[:, base_slot // 16: base_slot // 16 + ncols16])
                        stage = stagep.tile([128, cfg.nidx], BF16, tag="stage")
                        nc.gpsimd.dma_gather(
                            out_ap=stage[:, :(-(-n // 128)) * 128].rearrange(
                                "p (g e) -> p g e", e=D),
                            in_ap=table[ch * CH:(ch + 1) * CH, :],
                            idxs_ap=idx_sb_t[:, :ncols16],
                            num_idxs=n, num_idxs_reg=n, elem_size=D,
                            single_packet=False, queue_num=qn[0] % cfg.gq)
                        qn[0] += 1
                        nu = len(units)
                        oh = ohp.tile([128, MAXU * 128], BF16, tag="oh")
                        c0 = units[0][2]
                        assert units[-1][2] - c0 + 1 == nu
                        nc.vector.tensor_tensor(
                            out=oh[:, :nu * 128].rearrange("p (u e) -> p u e", e=128),
                            in0=iota_bf[:].rearrange("p (a e) -> p a e", a=1)
                                .to_broadcast([128, nu, 128]),
                            in1=dstl_sb[:, c0:c0 + nu].to_broadcast([128, nu, 128]),
                            op=mybir.AluOpType.is_equal)
                        stage3 = stage[:].rearrange("p (g e) -> p g e", e=D)
                        for (g, blk, col, st, sp) in units:
                            gi = (blk - blk0) // 4
                            sl = (blk - blk0) % 4
                            kk = min(128, n - g * 128)
                            nc.tensor.matmul(
                                out=gtiles[gi][:, sl * 128:(sl + 1) * 128],
                                lhsT=stage3[:kk, g, :],
                                rhs=oh[:kk, (col - c0) * 128:(col - c0 + 1) * 128],
                                start=False, stop=False, skip_group_check=True)
                        ii += 1
                    for gt in gtiles:
                        nc.tensor.matmul(out=gt[:], lhsT=iota_bf[:], rhs=zeros512[:],
                                         start=False, stop=True)
                    # final pass per group + per-block dense work
                    for gi in range(ngrp):
                        w = min(4, sbn - gi * 4)
                        hT = hTp.tile([128, 512], BF16, tag="hT")
                        dslice = dv[:, gi * 4 * 128:(gi * 4 + w) * 128]
                        if layer == 0:
                            nc.vector.tensor_tensor(
                                out=hT[:, :w * 128], in0=gtiles[gi][:, :w * 128],
                                in1=dslice, op=mybir.AluOpType.mult)
                        else:
                            g2f = hTp.tile([128, 512], F32, tag="g2f")
                            nc.vector.tensor_tensor(
                                out=g2f[:, :w * 128], in0=gtiles[gi][:, :w * 128],
                                in1=dslice, op=mybir.AluOpType.mult)
                            nc.scalar.activation(
                                hT[:, :w * 128], g2f[:, :w * 128],
                                mybir.ActivationFunctionType.Identity,
                                bias=bias_sb[:, 5:6], scale=1.0)
                        for k in range(w):
                            b = blk0 + gi * 4 + k
                            hTb = hT[:, k * 128:(k + 1) * 128]
                            if layer == 0:
                                # g1T halves
                                g1ps = pdensep.tile([128, 512], F32, tag="pd")
                                for h in range(2):
                                    nc.tensor.matmul(
                                        out=g1ps[:, h * 128:(h + 1) * 128],
                                        lhsT=wg1_sb[:, h * 128:(h + 1) * 128],
                                        rhs=hTb, start=True, stop=True)
                                g1b = smp.tile([128, 256], BF16, tag="g1b")
                                for h in range(2):
                                    nc.scalar.activation(
                                        g1b[:, h * 128:(h + 1) * 128],
                                        g1ps[:, h * 128:(h + 1) * 128],
                                        mybir.ActivationFunctionType.Identity,
                                        bias=bias_sb[:, 3 + h:4 + h], scale=1.0)
                                for h in range(2):
                                    nc.sync.dma_start(
                                        out=g1T_d[h][:, b * 128:(b + 1) * 128],
                                        in_=g1b[:, h * 128:(h + 1) * 128])
                                # z block (node-major): lhsT = g1T half, rhs = Wg2 half
                                zps = pdensep.tile([128, 512], F32, tag="pd")
                                for h in range(2):
                                    nc.tensor.matmul(
                                        out=zps[:, :128],
                                        lhsT=g1b[:, h * 128:(h + 1) * 128],
                                        rhs=wg2_sb[h][:],
                                        start=(h == 0), stop=(h == 1))
                                zb = smp.tile([128, 128], BF16, tag="zb")
                                nc.vector.tensor_scalar_mul(
                                    out=zb[:], in0=zps[:, :128],
                                    scalar1=dinvc_sb[:, b:b + 1])
                                nc.sync.dma_start(
                                    out=zsh[b * 128:(b + 1) * 128, :], in_=zb[:])
                                # z block transposed (feature-major) for the
                                # layer-2 self-loop psum add
                                zTps = pdensep.tile([128, 512], F32, tag="pd")
                                for h in range(2):
                                    nc.tensor.matmul(
                                        out=zTps[:, :128],
                                        lhsT=wg2_sb[h][:],
                                        rhs=g1b[:, h * 128:(h + 1) * 128],
                                        start=(h == 0), stop=(h == 1))
                                zTb = smp.tile([128, 128], BF16, tag="zTb")
                                nc.vector.tensor_tensor(
                                    out=zTb[:], in0=zTps[:, :128],
                                    in1=dv[:, (gi * 4 + k) * 128:(gi * 4 + k + 1) * 128],
                                    op=mybir.AluOpType.mult)
                                nc.sync.dma_start(
                                    out=zTf_d[:, b * 128:(b + 1) * 128], in_=zTb[:])
                            else:
                                # x2 = W_out^T @ [xn; g1; g2]
                                xnb = smp.tile([128, 128], BF16, tag="xnb")
                                nc.scalar.dma_start(
                                    out=xnb[:], in_=xnT[:, b * 128:(b + 1) * 128])
                                g1b0 = smp.tile([128, 128], BF16, tag="g1r0")
                                g1b1 = smp.tile([128, 128], BF16, tag="g1r1")
                                nc.scalar.dma_start(
                                    out=g1b0[:], in_=g1T_d[0][:, b * 128:(b + 1) * 128])
                                nc.scalar.dma_start(
                                    out=g1b1[:], in_=g1T_d[1][:, b * 128:(b + 1) * 128])
                                xps = pdensep.tile([128, 512], F32, tag="pd")
                                nc.tensor.matmul(out=xps[:, :128], lhsT=wout_sb[0][:],
                                                 rhs=xnb[:], start=True, stop=False)
                                nc.tensor.matmul(out=xps[:, :128], lhsT=wout_sb[1][:],
                                                 rhs=g1b0[:], start=False, stop=False)
                                nc.tensor.matmul(out=xps[:, :128], lhsT=wout_sb[2][:],
                                                 rhs=g1b1[:], start=False, stop=False)
                                nc.tensor.matmul(out=xps[:, :128], lhsT=wout_sb[3][:],
                                                 rhs=hTb, start=False, stop=True)
                                x2b = outsp.tile([128, 128], F32, tag="x2b")
                                nc.scalar.activation(
                                    x2b[:], xps[:, :128],
                                    mybir.ActivationFunctionType.Identity,
                                    bias=bias_sb[:, 6:7], scale=1.0)
                                nc.sync.dma_start(
                                    out=x2_out[:, b * 128:(b + 1) * 128], in_=x2b[:])

            # dstl resident
            dstl_sb = constp.tile([128, sched["col_count"]], BF16, tag="dstl")
            nc.sync.dma_start(out=dstl_sb[:], in_=dstl_d[:, :])

            # per-queue DMA completion semaphores for prepare_only gathers
            dma_sems = [nc.alloc_semaphore(f"gsem{q}") for q in range(cfg.gq)]

            agg_layer(utable, uTf, 0)

            # AllGather z table
            nc.gpsimd.collective_compute(
                "AllGather", mybir.AluOpType.bypass,
                ins=[zsh.opt()],
                outs=[ztable[:cfg.trows, :].opt()],
                replica_groups=[list(range(cfg.ncores))],
            )

            agg_layer(ztable, zTf_d, 1)

    nc.compile()
    return nc


def make_inmaps(cfg, sched, inputs):
    K, PPC = cfg.ncores, cfg.ppc
    x_self = np.asarray(inputs["x_self"], np.float32)
    x_nb = np.asarray(inputs["x_neighbor"], np.float32)
    dinv = sched["dinv"]
    core_of, pos_of = sched["core_of"], sched["pos_of"]

    bf = ml_dtypes.bfloat16
    w = {k: np.asarray(inputs[k], np.float32) for k in
         ("W_in_self", "W_out_self", "Wg1", "Wg2", "W_out")}
    biases = np.zeros((128, 7), np.float32)
    biases[:, 0] = inputs["b_in_self"][:128]
    biases[:, 1] = inputs["b_in_self"][128:]
    biases[:, 2] = inputs["b_out_self"]
    biases[:, 3] = inputs["bg1"][:128]
    biases[:, 4] = inputs["bg1"][128:]
    biases[:, 5] = inputs["bg2"]
    biases[:, 6] = inputs["b_out"]

    # full replicated u table (bf16), rows laid out core-major
    ut_full = np.zeros((cfg.trows_pad if hasattr(cfg, 'trows_pad') else 0, 128),
                       np.float32)
    row_of = sched["core_of"] * PPC + sched["pos_of"]
    ut_full[row_of] = x_nb * dinv[:, None]
    ut_full = ut_full.astype(bf)

    in_maps = []
    for c in range(K):
        sel = core_of == c
        nodes = np.where(sel)[0]
        pos = pos_of[sel]
        xnT = np.zeros((128, PPC), np.float32)
        xnT[:, pos] = x_nb[nodes].T
        xsT = np.zeros((128, PPC), np.float32)
        xsT[:, pos] = x_self[nodes].T
        dv = np.zeros(PPC, np.float32)
        dv[pos] = dinv[nodes]
        dinvb = np.broadcast_to(dv[None, :], (128, PPC)).copy()
        dinvc = dv.reshape(cfg.nb, 128).T.copy()
        uTf_c = np.zeros((128, PPC), np.float32)
        uTf_c[:, pos] = (x_nb[nodes] * dinv[nodes, None]).T
        in_maps.append({
            "utable": ut_full,
            "uTf": uTf_c.astype(bf),
            "xnT": xnT.astype(bf),
            "xsT": xsT.astype(bf),
            "dinvb": dinvb,
            "dinvc": dinvc,
            "idx": sched["idx_arr"][c],
            "dstl": sched["dstl_cols"][c].astype(bf),
            "W_in_self": w["W_in_self"].astype(bf),
            "W_out_self": w["W_out_self"].astype(bf),
            "Wg1": w["Wg1"].astype(bf),
            "Wg2": w["Wg2"].astype(bf),
            "W_out": w["W_out"].astype(bf),
            "biases": biases,
        })
    return in_maps


def unshard(cfg, sched, results):
    N = cfg.N
    l1 = np.zeros((N, 128), np.float32)
    x2 = np.zeros((N, 128), np.float32)
    core_of, pos_of = sched["core_of"], sched["pos_of"]
    for c in range(cfg.ncores):
        sel = core_of == c
        nodes = np.where(sel)[0]
        pos = pos_of[sel]
        l1[nodes] = results[c]["l1T"].T[pos]
        x2[nodes] = results[c]["x2T"].T[pos]
    return l1, x2


def kernel(**inputs):
    cfg = CFG(N=inputs["x_self"].shape[0])
    sched = build_schedule(np.asarray(inputs["edge_index"]), cfg)
    nc = build_program(cfg, sched)
    in_maps = make_inmaps(cfg, sched, inputs)
    res = bass_utils.run_bass_kernel_spmd(
        nc, in_maps, core_ids=list(range(cfg.ncores)))
    l1, x2 = unshard(cfg, sched, res.results)
    return (l1, x2)



# revision 37
# speedup vs baseline: 1.4063x; 1.1111x over previous
"""Trainium2 Bass kernel for nn_LinearEncoder (2-layer GCN + dense branch).

Strategy (8 NeuronCores, SPMD):
  - Nodes are degree-sorted and dealt round-robin to 8 cores (load balance);
    each core owns PPC=12544 destination positions (98 blocks of 128).
  - GCN linearity: aggregate the 128-wide scaled node table u = x*dinv
    (resp. z = (g1@Wg2)*dinv for layer 2), then apply the weight matmul once.
  - Layer-1 table u is host-computed and replicated to every core as an
    input (no AllGather); the layer-2 z table is computed on-device and
    AllGather'd (bf16). Each core gathers its incoming-edge source rows
    with batched dma_gather (4096-idx instructions, int16 chunk-relative
    indices, 4 SWDGE queues) and segment-sums them on the TensorEngine via
    one-hot matmuls accumulated in PSUM (feat-major), superblock by
    superblock. Self-loops skip the gather: they are added straight into
    PSUM by identity-matrix matmuls over the local (dinv-scaled) table.
  - Dense branches are feat-major bf16 matmuls with biases/ReLU on the
    Scalar (ACT) engine; outputs are written transposed per core and
    un-permuted on the host.
"""

import numpy as np
import ml_dtypes

import concourse.bacc as bacc
import concourse.mybir as mybir
import concourse.tile as tile
from concourse import bass_utils

F32 = mybir.dt.float32
BF16 = mybir.dt.bfloat16
I16 = mybir.dt.int16
I32 = mybir.dt.int32
NEG = -1.0  # dstl mask value


class CFG:
    def __init__(self, N, ncores=8, ch_rows=32768, nidx=4096, sb_blocks=20):
        self.N = N
        self.ncores = ncores
        per = -(-N // ncores)
        self.per = per                      # real nodes per core (first cores)
        self.ppc = -(-per // 128) * 128     # padded per core
        self.nb = self.ppc // 128           # blocks per core
        self.trows = ncores * self.ppc      # real table rows
        self.ch = ch_rows
        self.nchunk = -(-self.trows // ch_rows)
        self.trows_pad = self.nchunk * ch_rows
        self.nidx = nidx
        self.gq = 4  # gather SWDGE queues (set 1 for CoreSim validation)
        # superblock partition of blocks
        sbs = []
        b = self.nb
        while b > 0:
            sbs.append(min(sb_blocks, b))
            b -= min(sb_blocks, b)
        self.sbs = sbs


def _deal_nodes(deg, cfg):
    """Degree-sorted round-robin deal of nodes to (core, pos)."""
    N = cfg.N
    order = np.argsort(-deg, kind="stable")
    core_of = np.empty(N, np.int64)
    pos_of = np.empty(N, np.int64)
    r = np.arange(N, dtype=np.int64)
    core_of[order] = r % cfg.ncores
    pos_of[order] = r // cfg.ncores
    return core_of, pos_of


def build_schedule(edge_index, cfg):
    """Static SPMD schedule + per-core device arrays, from the actual graph."""
    N, K = cfg.N, cfg.ncores
    src = np.asarray(edge_index[0], dtype=np.int64)
    dst = np.asarray(edge_index[1], dtype=np.int64)
    deg = np.bincount(dst, minlength=N).astype(np.int64) + 1
    dinv = (1.0 / np.sqrt(deg.astype(np.float64))).astype(np.float32)

    core_of, pos_of = _deal_nodes(deg, cfg)
    row_of = core_of * cfg.ppc + pos_of  # table row of each node

    # per-core real node counts
    npercore = np.bincount(core_of, minlength=K)

    # real edges only; self-loops are added on-device via identity matmuls
    esrc = src
    edst = dst

    e_core = core_of[edst]
    e_pos = pos_of[edst]
    e_blk = e_pos >> 7
    e_p = (e_pos & 127).astype(np.float32)
    e_rowsrc = row_of[esrc]
    e_ch = e_rowsrc // cfg.ch
    e_rel = (e_rowsrc % cfg.ch).astype(np.int16)

    nb, nch = cfg.nb, cfg.nchunk
    sb_of_blk = np.repeat(np.arange(len(cfg.sbs)), cfg.sbs)

    # group = (sb, ch, blk); order edges by (core, sb, ch, blk)
    g_of_e = (sb_of_blk[e_blk] * nch + e_ch) * nb + e_blk  # group id within core
    ngrp_ids = len(cfg.sbs) * nch * nb  # sparse (blk implies sb) but fine
    key = e_core * ngrp_ids + g_of_e
    eord = np.argsort(key, kind="stable")
    key_s = key[eord]

    # counts per (core, group)
    cnt = np.bincount(key_s, minlength=K * ngrp_ids).reshape(K, ngrp_ids)

    # group list in slot order: for sb, for ch, for blk in sb
    grp_list = []  # (sb, ch, blk, gid)
    for sbi, sbn in enumerate(cfg.sbs):
        blk0 = sum(cfg.sbs[:sbi])
        for ch in range(nch):
            for blk in range(blk0, blk0 + sbn):
                gid = (sbi * nch + ch) * nb + blk
                grp_list.append((sbi, ch, blk, gid))

    # padded group sizes: max over cores; chunk-0 groups at least 1
    gmax = {}
    for sbi, ch, blk, gid in grp_list:
        m = int(cnt[:, gid].max())
        if ch == 0:
            m = max(m, 1)
        gmax[gid] = m

    # per-(sb,ch) runs: pad total to x16; compute slot offsets
    runs = []  # (sbi, ch, slot_off, n_slots, [(blk, gid, off_in_run, gsize)])
    slot_blk_parts = []
    total = 0
    for sbi, sbn in enumerate(cfg.sbs):
        blk0 = sum(cfg.sbs[:sbi])
        for ch in range(nch):
            glist = []
            off = 0
            for blk in range(blk0, blk0 + sbn):
                gid = (sbi * nch + ch) * nb + blk
                gs = gmax[gid]
                if gs:
                    glist.append((blk, gid, off, gs))
                off += gs
            pad_tail = (-off) % 16
            n = off + pad_tail
            sb_slot_blk = np.full(n, -1, np.int64)
            for blk, gid, o, gs in glist:
                sb_slot_blk[o:o + gs] = blk
            runs.append((sbi, ch, total, n, glist))
            slot_blk_parts.append(sb_slot_blk)
            total += n
    n_slots = total
    slot_blk = np.concatenate(slot_blk_parts) if slot_blk_parts else np.zeros(0, np.int64)

    # pad rows per chunk (zero rows of the table): first padded position of
    # some core inside each chunk's row range
    pad_row_rel = np.full(nch, -1, np.int64)
    for c in range(K):
        if npercore[c] < cfg.ppc:
            r0 = c * cfg.ppc + npercore[c]
            ch = r0 // cfg.ch
            if pad_row_rel[ch] < 0:
                pad_row_rel[ch] = r0 % cfg.ch
    # fallback: fill missing chunks with any real zero... must not happen
    for ch in range(nch):
        if pad_row_rel[ch] < 0:
            # point at the last real row of the chunk; its value times a
            # zero one-hot column contributes nothing (dstl = -1 for pads)
            pad_row_rel[ch] = 0

    # per-core slot arrays: idx (int16 rel) + dstl (float p or -1)
    slot_idx = np.zeros((K, n_slots), np.int16)
    slot_dstl = np.full((K, n_slots), NEG, np.float32)
    # default pad idx per run
    for (sbi, ch, off, n, glist) in runs:
        slot_idx[:, off:off + n] = pad_row_rel[ch]
    # place real edges: rank within (core, group)
    grp_off = {}
    for (sbi, ch, off, n, glist) in runs:
        for blk, gid, o, gs in glist:
            grp_off[gid] = off + o
    # vectorized placement
    uk, inv = np.unique(key_s, return_inverse=True)
    starts = np.searchsorted(key_s, uk)
    rank = np.arange(len(key_s)) - starts[inv]
    core_s = key_s // ngrp_ids
    gid_s = key_s % ngrp_ids
    base = np.array([grp_off.get(int(g), -1) for g in uk % ngrp_ids], np.int64)
    slot_pos = base[inv] + rank
    assert (base[inv] >= 0).all()
    slot_idx[core_s, slot_pos] = e_rel[eord]
    slot_dstl[core_s, slot_pos] = e_p[eord]

    # instructions: slice each run into <= nidx pieces
    instrs = []  # (sbi, ch, idx_col_off, n_idx, units)
    # units: (g_tile, grp4, blk, col, start, stop)
    # dstl columns are appended as discovered
    col_count = 0
    unit_cols = []  # (slot_off_of_tile, n_valid, blk) for building dstl cols
    first_seen = {}
    last_seen = {}
    order_units = []
    for (sbi, ch, roff, rn, glist) in runs:
        o = 0
        while o < rn:
            n = min(cfg.nidx, rn - o)
            base_slot = roff + o
            units = []
            ntiles = -(-n // 128)
            for g in range(ntiles):
                t0 = base_slot + g * 128
                t1 = min(t0 + 128, base_slot + n)
                blks = np.unique(slot_blk[t0:t1])
                for blk in blks:
                    if blk < 0:
                        continue
                    col = col_count
                    col_count += 1
                    unit_cols.append((t0, t1 - t0, int(blk)))
                    u = [g, int(blk), col]
                    units.append(u)
                    kkey = (sbi, int(blk))
                    if kkey not in first_seen:
                        first_seen[kkey] = (len(instrs), len(units) - 1)
                    last_seen[kkey] = (len(instrs), len(units) - 1)
            instrs.append([sbi, ch, base_slot, n, units])
            o += n
    # mark start/stop
    for ii, (sbi, ch, base_slot, n, units) in enumerate(instrs):
        for ui, (g, blk, col) in enumerate(units):
            st = first_seen[(sbi, blk)] == (ii, ui)
            sp = last_seen[(sbi, blk)] == (ii, ui)
            units[ui] = (g, blk, col, st, sp)

    # per-core dstl column array [128, col_count]
    dstl_cols = np.full((K, 128, col_count), NEG, np.float32)
    for col, (t0, nvalid, blk) in enumerate(unit_cols):
        seg = slot_dstl[:, t0:t0 + nvalid]           # [K, nvalid]
        segblk = slot_blk[t0:t0 + nvalid]            # [nvalid]
        m = segblk == blk
        v = np.where(m[None, :], seg, NEG)
        dstl_cols[:, :nvalid, col] = v

    # per-core idx array wrapped per instruction: [128, n_slots/16]
    assert n_slots % 16 == 0
    idx_arr = np.zeros((K, 128, n_slots // 16), np.int16)
    for (sbi, ch, base_slot, n, units) in instrs:
        seg = slot_idx[:, base_slot:base_slot + n]    # [K, n]
        assert n % 16 == 0
        w = seg.reshape(K, n // 16, 16).transpose(0, 2, 1)  # [K,16,n/16]
        c0 = base_slot // 16
        for grp in range(8):
            idx_arr[:, 16 * grp:16 * grp + 16, c0:c0 + n // 16] = w

    return dict(
        deg=deg, dinv=dinv, core_of=core_of, pos_of=pos_of, row_of=row_of,
        npercore=npercore, instrs=instrs, n_slots=n_slots, col_count=col_count,
        idx_arr=idx_arr, dstl_cols=dstl_cols, runs=runs,
    )


def build_program(cfg, sched):
    nc = bacc.Bacc("TRN2", target_bir_lowering=False, debug=False,
                   num_devices=cfg.ncores, num_swdge_queues=cfg.gq)
    PPC, NB, CH, NCH = cfg.ppc, cfg.nb, cfg.ch, cfg.nchunk
    D = 128

    # ---- inputs ----
    utable = nc.dram_tensor("utable", [cfg.trows_pad, D], BF16, kind="ExternalInput")
    uTf = nc.dram_tensor("uTf", [D, PPC], BF16, kind="ExternalInput")
    xnT = nc.dram_tensor("xnT", [D, PPC], BF16, kind="ExternalInput")
    xsT = nc.dram_tensor("xsT", [D, PPC], BF16, kind="ExternalInput")
    dinvb = nc.dram_tensor("dinvb", [D, PPC], F32, kind="ExternalInput")
    dinvc = nc.dram_tensor("dinvc", [D, NB], F32, kind="ExternalInput")
    idx_d = nc.dram_tensor("idx", [D, sched["n_slots"] // 16], I16, kind="ExternalInput")
    dstl_d = nc.dram_tensor("dstl", [D, sched["col_count"]], BF16, kind="ExternalInput")
    w_ins = nc.dram_tensor("W_in_self", [D, 256], BF16, kind="ExternalInput")
    w_os = nc.dram_tensor("W_out_self", [384, D], BF16, kind="ExternalInput")
    wg1 = nc.dram_tensor("Wg1", [D, 256], BF16, kind="ExternalInput")
    wg2 = nc.dram_tensor("Wg2", [256, D], BF16, kind="ExternalInput")
    w_out = nc.dram_tensor("W_out", [512, D], BF16, kind="ExternalInput")
    biases = nc.dram_tensor("biases", [D, 7], F32, kind="ExternalInput")
    # bias cols: 0,1 b_in_self | 2 b_out_self | 3,4 bg1 | 5 bg2 | 6 b_out
    l1_out = nc.dram_tensor("l1T", [D, PPC], F32, kind="ExternalOutput")
    x2_out = nc.dram_tensor("x2T", [D, PPC], F32, kind="ExternalOutput")

    instrs = sched["instrs"]
    MAXU = max(len(u[4]) for u in instrs)

    with tile.TileContext(nc) as tc:
        with tc.tile_pool(name="const", bufs=1) as constp, \
             tc.tile_pool(name="dram", bufs=1, space="DRAM") as dramp, \
             tc.tile_pool(name="idxs", bufs=6) as idxsp, \
             tc.tile_pool(name="stage", bufs=8) as stagep, \
             tc.tile_pool(name="oh", bufs=4) as ohp, \
             tc.tile_pool(name="pagg", bufs=5, space="PSUM") as paggp, \
             tc.tile_pool(name="pdense", bufs=3, space="PSUM") as pdensep, \
             tc.tile_pool(name="hT", bufs=4) as hTp, \
             tc.tile_pool(name="sm", bufs=8) as smp, \
             tc.tile_pool(name="outs", bufs=4) as outsp, \
             tc.tile_pool(name="dinvs", bufs=2) as dinvsp:

            # constants
            iota_i32 = constp.tile([128, 128], I32)
            nc.gpsimd.iota(iota_i32[:], pattern=[[1, 128]], base=0, channel_multiplier=0)
            iota_bf = constp.tile([128, 128], BF16)
            nc.vector.tensor_copy(out=iota_bf[:], in_=iota_i32[:])
            zeros512 = constp.tile([128, 512], BF16)
            nc.vector.memset(zeros512[:], 0.0)
            # identity matrix for self-loop psum adds
            iota_ch = constp.tile([128, 128], I32)
            nc.gpsimd.iota(iota_ch[:], pattern=[[0, 128]], base=0, channel_multiplier=1)
            ident = constp.tile([128, 128], BF16)
            nc.vector.tensor_tensor(out=ident[:], in0=iota_i32[:], in1=iota_ch[:],
                                    op=mybir.AluOpType.is_equal)

            wins_sb = constp.tile([128, 256], BF16)
            nc.sync.dma_start(out=wins_sb[:], in_=w_ins[:, :])
            wos_sb = [constp.tile([128, 128], BF16, tag=f"wos{k}", name=f"wos{k}") for k in range(3)]
            for k in range(3):
                nc.sync.dma_start(out=wos_sb[k][:], in_=w_os[k * 128:(k + 1) * 128, :])
            wg1_sb = constp.tile([128, 256], BF16)
            nc.sync.dma_start(out=wg1_sb[:], in_=wg1[:, :])
            wg2_sb = [constp.tile([128, 128], BF16, tag=f"wg2{k}", name=f"wg2{k}") for k in range(2)]
            for k in range(2):
                nc.sync.dma_start(out=wg2_sb[k][:], in_=wg2[k * 128:(k + 1) * 128, :])
            wout_sb = [constp.tile([128, 128], BF16, tag=f"wo{k}", name=f"wo{k}") for k in range(4)]
            for k in range(4):
                nc.sync.dma_start(out=wout_sb[k][:], in_=w_out[k * 128:(k + 1) * 128, :])
            bias_sb = constp.tile([128, 7], F32)
            nc.sync.dma_start(out=bias_sb[:], in_=biases[:, :])
            dinvc_sb = constp.tile([128, NB], F32)
            nc.sync.dma_start(out=dinvc_sb[:], in_=dinvc[:, :])


            # z-table plumbing (written during L1 phase, AG'd before L2)
            zsh = dramp.tile([PPC, D], BF16)
            ztable = dramp.tile([cfg.trows_pad, D], BF16)
            zTf_d = dramp.tile([D, PPC], BF16)

            # g1T spill (two halves, feat-major)
            g1T_d = [dramp.tile([D, PPC], BF16, name=f"g1T{h}") for h in range(2)]

            # ---------- dense self branch (independent) ----------
            for b in range(NB):
                xs_blk = smp.tile([128, 128], BF16, tag="xs")
                nc.scalar.dma_start(out=xs_blk[:], in_=xsT[:, b * 128:(b + 1) * 128])
                l1ps = pdensep.tile([128, 512], F32, tag="pd")
                for h in range(2):
                    nc.tensor.matmul(
                        out=l1ps[:, h * 128:(h + 1) * 128],
                        lhsT=wins_sb[:, h * 128:(h + 1) * 128],
                        rhs=xs_blk[:], start=True, stop=True)
                l1b = smp.tile([128, 256], BF16, tag="l1b")
                for h in range(2):
                    nc.scalar.activation(
                        l1b[:, h * 128:(h + 1) * 128],
                        l1ps[:, h * 128:(h + 1) * 128],
                        mybir.ActivationFunctionType.Relu,
                        bias=bias_sb[:, h:h + 1], scale=1.0)
                o1ps = pdensep.tile([128, 512], F32, tag="pd")
                nc.tensor.matmul(out=o1ps[:, :128], lhsT=wos_sb[0][:], rhs=xs_blk[:],
                                 start=True, stop=False)
                for h in range(2):
                    nc.tensor.matmul(out=o1ps[:, :128], lhsT=wos_sb[1 + h][:],
                                     rhs=l1b[:, h * 128:(h + 1) * 128],
                                     start=False, stop=(h == 1))
                o1 = outsp.tile([128, 128], F32, tag="o1")
                nc.scalar.activation(
                    o1[:], o1ps[:, :128],
                    mybir.ActivationFunctionType.Identity,
                    bias=bias_sb[:, 2:3], scale=1.0)
                nc.sync.dma_start(out=l1_out[:, b * 128:(b + 1) * 128], in_=o1[:])

            # ---------- per-layer aggregation ----------
            def agg_layer(table, selfT_d, layer):
                """Returns nothing; layer==0 computes g1T+z, layer==1 computes g2T+x2."""
                qn = [0]
                ii = 0
                n_instr = len(instrs)
                while ii < n_instr:
                    sbi = instrs[ii][0]
                    blk0 = sum(cfg.sbs[:sbi])
                    sbn = cfg.sbs[sbi]
                    ngrp = -(-sbn // 4)
                    # psum group tiles for this superblock
                    gtiles = [paggp.tile([128, 512], F32, tag="agg", name=f"agg_{layer}_{sbi}_{gg}") for gg in range(ngrp)]
                    for gt in gtiles:
                        nc.tensor.matmul(out=gt[:], lhsT=iota_bf[:], rhs=zeros512[:],
                                         start=True, stop=False)
                    # dinvb slice for this superblock
                    dv = dinvsp.tile([128, sbn * 128], F32, tag="dv")
                    nc.sync.dma_start(
                        out=dv[:], in_=dinvb[:, blk0 * 128:(blk0 + sbn) * 128])
                    # self-loop contributions (identity matmul into psum)
                    sfT = dinvsp.tile([128, sbn * 128], BF16, tag="sfT")
                    nc.sync.dma_start(
                        out=sfT[:], in_=selfT_d[:, blk0 * 128:(blk0 + sbn) * 128])
                    for gi in range(ngrp):
                        w = min(4, sbn - gi * 4)
                        nc.tensor.matmul(
                            out=gtiles[gi][:, :w * 128], lhsT=ident[:],
                            rhs=sfT[:, gi * 512:gi * 512 + w * 128],
                            start=False, stop=False, skip_group_check=True)
                    # all instructions of this superblock (4 chunks)
                    while ii < n_instr and instrs[ii][0] == sbi:
                        _, ch, base_slot, n, units = instrs[ii]
                        idx_sb_t = idxsp.tile([128, -(-cfg.nidx // 16)], I16, tag="idx")
                        ncols16 = n // 16
                        nc.sync.dma_start(
                            out=idx_sb_t[:, :ncols16],
                            in_=idx_d[:, base_slot // 16: base_slot // 16 + ncols16])
                        stage = stagep.tile([128, cfg.nidx], BF16, tag="stage")
                        nc.gpsimd.dma_gather(
                            out_ap=stage[:, :(-(-n // 128)) * 128].rearrange(
                                "p (g e) -> p g e", e=D),
                            in_ap=table[ch * CH:(ch + 1) * CH, :],
                            idxs_ap=idx_sb_t[:, :ncols16],
                            num_idxs=n, num_idxs_reg=n, elem_size=D,
                            single_packet=False, queue_num=qn[0] % cfg.gq)
                        qn[0] += 1
                        nu = len(units)
                        oh = ohp.tile([128, MAXU * 128], BF16, tag="oh")
                        c0 = units[0][2]
                        assert units[-1][2] - c0 + 1 == nu
                        nc.vector.tensor_tensor(
                            out=oh[:, :nu * 128].rearrange("p (u e) -> p u e", e=128),
                            in0=iota_bf[:].rearrange("p (a e) -> p a e", a=1)
                                .to_broadcast([128, nu, 128]),
                            in1=dstl_sb[:, c0:c0 + nu].to_broadcast([128, nu, 128]),
                            op=mybir.AluOpType.is_equal)
                        stage3 = stage[:].rearrange("p (g e) -> p g e", e=D)
                        for (g, blk, col, st, sp) in units:
                            gi = (blk - blk0) // 4
                            sl = (blk - blk0) % 4
                            kk = min(128, n - g * 128)
                            nc.tensor.matmul(
                                out=gtiles[gi][:, sl * 128:(sl + 1) * 128],
                                lhsT=stage3[:kk, g, :],
                                rhs=oh[:kk, (col - c0) * 128:(col - c0 + 1) * 128],
                                start=False, stop=False, skip_group_check=True)
                        ii += 1
                    for gt in gtiles:
                        nc.tensor.matmul(out=gt[:], lhsT=iota_bf[:], rhs=zeros512[:],
                                         start=False, stop=True)
                    # final pass per group + per-block dense work
                    for gi in range(ngrp):
                        w = min(4, sbn - gi * 4)
                        hT = hTp.tile([128, 512], BF16, tag="hT")
                        dslice = dv[:, gi * 4 * 128:(gi * 4 + w) * 128]
                        if layer == 0:
                            nc.vector.tensor_tensor(
                                out=hT[:, :w * 128], in0=gtiles[gi][:, :w * 128],
                                in1=dslice, op=mybir.AluOpType.mult)
                        else:
                            g2f = hTp.tile([128, 512], F32, tag="g2f")
                            nc.vector.tensor_tensor(
                                out=g2f[:, :w * 128], in0=gtiles[gi][:, :w * 128],
                                in1=dslice, op=mybir.AluOpType.mult)
                            nc.scalar.activation(
                                hT[:, :w * 128], g2f[:, :w * 128],
                                mybir.ActivationFunctionType.Identity,
                                bias=bias_sb[:, 5:6], scale=1.0)
                        for k in range(w):
                            b = blk0 + gi * 4 + k
                            hTb = hT[:, k * 128:(k + 1) * 128]
                            if layer == 0:
                                # g1T halves
                                g1ps = pdensep.tile([128, 512], F32, tag="pd")
                                for h in range(2):
                                    nc.tensor.matmul(
                                        out=g1ps[:, h * 128:(h + 1) * 128],
                                        lhsT=wg1_sb[:, h * 128:(h + 1) * 128],
                                        rhs=hTb, start=True, stop=True)
                                g1b = smp.tile([128, 256], BF16, tag="g1b")
                                for h in range(2):
                                    nc.scalar.activation(
                                        g1b[:, h * 128:(h + 1) * 128],
                                        g1ps[:, h * 128:(h + 1) * 128],
                                        mybir.ActivationFunctionType.Identity,
                                        bias=bias_sb[:, 3 + h:4 + h], scale=1.0)
                                for h in range(2):
                                    nc.sync.dma_start(
                                        out=g1T_d[h][:, b * 128:(b + 1) * 128],
                                        in_=g1b[:, h * 128:(h + 1) * 128])
                                # z block (node-major): lhsT = g1T half, rhs = Wg2 half
                                zps = pdensep.tile([128, 512], F32, tag="pd")
                                for h in range(2):
                                    nc.tensor.matmul(
                                        out=zps[:, :128],
                                        lhsT=g1b[:, h * 128:(h + 1) * 128],
                                        rhs=wg2_sb[h][:],
                                        start=(h == 0), stop=(h == 1))
                                zb = smp.tile([128, 128], BF16, tag="zb")
                                nc.vector.tensor_scalar_mul(
                                    out=zb[:], in0=zps[:, :128],
                                    scalar1=dinvc_sb[:, b:b + 1])
                                nc.sync.dma_start(
                                    out=zsh[b * 128:(b + 1) * 128, :], in_=zb[:])
                                # z block transposed (feature-major) for the
                                # layer-2 self-loop psum add
                                zTps = pdensep.tile([128, 512], F32, tag="pd")
                                for h in range(2):
                                    nc.tensor.matmul(
                                        out=zTps[:, :128],
                                        lhsT=wg2_sb[h][:],
                                        rhs=g1b[:, h * 128:(h + 1) * 128],
                                        start=(h == 0), stop=(h == 1))
                                zTb = smp.tile([128, 128], BF16, tag="zTb")
                                nc.vector.tensor_tensor(
                                    out=zTb[:], in0=zTps[:, :128],
                                    in1=dv[:, (gi * 4 + k) * 128:(gi * 4 + k + 1) * 128],
                                    op=mybir.AluOpType.mult)
                                nc.sync.dma_start(
                                    out=zTf_d[:, b * 128:(b + 1) * 128], in_=zTb[:])
                            else:
                                # x2 = W_out^T @ [xn; g1; g2]
                                xnb = smp.tile([128, 128], BF16, tag="xnb")
                                nc.scalar.dma_start(
                                    out=xnb[:], in_=xnT[:, b * 128:(b + 1) * 128])
                                g1b0 = smp.tile([128, 128], BF16, tag="g1r0")
                                g1b1 = smp.tile([128, 128], BF16, tag="g1r1")
                                nc.scalar.dma_start(
                                    out=g1b0[:], in_=g1T_d[0][:, b * 128:(b + 1) * 128])
                                nc.scalar.dma_start(
                                    out=g1b1[:], in_=g1T_d[1][:, b * 128:(b + 1) * 128])
                                xps = pdensep.tile([128, 512], F32, tag="pd")
                                nc.tensor.matmul(out=xps[:, :128], lhsT=wout_sb[0][:],
                                                 rhs=xnb[:], start=True, stop=False)
                                nc.tensor.matmul(out=xps[:, :128], lhsT=wout_sb[1][:],
                                                 rhs=g1b0[:], start=False, stop=False)
                                nc.tensor.matmul(out=xps[:, :128], lhsT=wout_sb[2][:],
                                                 rhs=g1b1[:], start=False, stop=False)
                                nc.tensor.matmul(out=xps[:, :128], lhsT=wout_sb[3][:],
                                                 rhs=hTb, start=False, stop=True)
                                x2b = outsp.tile([128, 128], F32, tag="x2b")
                                nc.scalar.activation(
                                    x2b[:], xps[:, :128],
                                    mybir.ActivationFunctionType.Identity,
                                    bias=bias_sb[:, 6:7], scale=1.0)
                                nc.sync.dma_start(
                                    out=x2_out[:, b * 128:(b + 1) * 128], in_=x2b[:])

            # dstl resident
            dstl_sb = constp.tile([128, sched["col_count"]], BF16, tag="dstl")
            nc.sync.dma_start(out=dstl_sb[:], in_=dstl_d[:, :])

            # per-queue DMA completion semaphores for prepare_only gathers
            dma_sems = [nc.alloc_semaphore(f"gsem{q}") for q in range(cfg.gq)]

            agg_layer(utable, uTf, 0)

            # AllGather z table
            nc.gpsimd.collective_compute(
                "AllGather", mybir.AluOpType.bypass,
                ins=[zsh.opt()],
                outs=[ztable[:cfg.trows, :].opt()],
                replica_groups=[list(range(cfg.ncores))],
            )

            agg_layer(ztable, zTf_d, 1)

    nc.compile()
    return nc


def make_inmaps(cfg, sched, inputs):
    K, PPC = cfg.ncores, cfg.ppc
    x_self = np.asarray(inputs["x_self"], np.float32)
    x_nb = np.asarray(inputs["x_neighbor"], np.float32)
    dinv = sched["dinv"]
    core_of, pos_of = sched["core_of"], sched["pos_of"]

    bf = ml_dtypes.bfloat16
    w = {k: np.asarray(inputs[k], np.float32) for k in
         ("W_in_self", "W_out_self", "Wg1", "Wg2", "W_out")}
    biases = np.zeros((128, 7), np.float32)
    biases[:, 0] = inputs["b_in_self"][:128]
    biases[:, 1] = inputs["b_in_self"][128:]
    biases[:, 2] = inputs["b_out_self"]
    biases[:, 3] = inputs["bg1"][:128]
    biases[:, 4] = inputs["bg1"][128:]
    biases[:, 5] = inputs["bg2"]
    biases[:, 6] = inputs["b_out"]

    # full replicated u table (bf16), rows laid out core-major
    ut_full = np.zeros((cfg.trows_pad if hasattr(cfg, 'trows_pad') else 0, 128),
                       np.float32)
    row_of = sched["core_of"] * PPC + sched["pos_of"]
    ut_full[row_of] = x_nb * dinv[:, None]
    ut_full = ut_full.astype(bf)

    in_maps = []
    for c in range(K):
        sel = core_of == c
        nodes = np.where(sel)[0]
        pos = pos_of[sel]
        xnT = np.zeros((128, PPC), np.float32)
        xnT[:, pos] = x_nb[nodes].T
        xsT = np.zeros((128, PPC), np.float32)
        xsT[:, pos] = x_self[nodes].T
        dv = np.zeros(PPC, np.float32)
        dv[pos] = dinv[nodes]
        dinvb = np.broadcast_to(dv[None, :], (128, PPC)).copy()
        dinvc = dv.reshape(cfg.nb, 128).T.copy()
        uTf_c = np.zeros((128, PPC), np.float32)
        uTf_c[:, pos] = (x_nb[nodes] * dinv[nodes, None]).T
        in_maps.append({
            "utable": ut_full,
            "uTf": uTf_c.astype(bf),
            "xnT": xnT.astype(bf),
            "xsT": xsT.astype(bf),
            "dinvb": dinvb,
            "dinvc": dinvc,
            "idx": sched["idx_arr"][c],
            "dstl": sched["dstl_cols"][c].astype(bf),
            "W_in_self": w["W_in_self"].astype(bf),
            "W_out_self": w["W_out_self"].astype(bf),
            "Wg1": w["Wg1"].astype(bf),
            "Wg2": w["Wg2"].astype(bf),
            "W_out": w["W_out"].astype(bf),
            "biases": biases,
        })
    return in_maps


def unshard(cfg, sched, results):
    N = cfg.N
    l1 = np.zeros((N, 128), np.float32)
    x2 = np.zeros((N, 128), np.float32)
    core_of, pos_of = sched["core_of"], sched["pos_of"]
    for c in range(cfg.ncores):
        sel = core_of == c
        nodes = np.where(sel)[0]
        pos = pos_of[sel]
        l1[nodes] = results[c]["l1T"].T[pos]
        x2[nodes] = results[c]["x2T"].T[pos]
    return l1, x2


def kernel(**inputs):
    cfg = CFG(N=inputs["x_self"].shape[0])
    sched = build_schedule(np.asarray(inputs["edge_index"]), cfg)
    nc = build_program(cfg, sched)
    in_maps = make_inmaps(cfg, sched, inputs)
    res = bass_utils.run_bass_kernel_spmd(
        nc, in_maps, core_ids=list(range(cfg.ncores)))
    l1, x2 = unshard(cfg, sched, res.results)
    return (l1, x2)

